# revision 29
# baseline (speedup 1.0000x reference)
"""Multi-head attention (B=8, S=1500, E=1024, H=16, D=64) on 8 trn2 NeuronCores.

Sharding: pure data-parallel over batch — core b computes batch element b
end-to-end (no collectives). Host pre-transposes x and the weights so every
device-side matmul has its contraction dim on the SBUF partition axis, and
folds the 1/sqrt(D) scale into Wq/bq and the V-bias into the output bias
(bo_eff = bo + Wo @ bv), so the device kernel never touches bv.

Device pipeline per core (all f32 storage, matmuls run as float32r):
  QT = (Wq/8)^T-proj of x^T   [1024, 1500]  (f-on-partition; bias bq/8 per-partition)
  KT = Wk^T-proj              [1024, 1500]
  V_aug = x @ Wv^T with a ones-column appended per head  [1500, 16*65]
  per (i-chunk, head): scoresT[j, i] via matmul(lhsT=KT_h, rhs=QT_h);
    exp on ACT (no max-subtraction: |scores| <~ 30, safely inside f32);
    causal masking structurally (affine_select zero-fill on diagonal blocks)
    or via an additive mask tensor (general path);
  out^T + rowsums in ONE matmul: lhsT=[V_h | 1] (65 cols), rhs=attnT;
  normalize: recip of the sums row, rank-1 matmul broadcast across 64
    partitions, multiply on eviction -> AO^T;
  yT = Wo^T-proj of AO^T + bo_eff  -> DRAM [1024, 1500], host transposes back.
"""

import sys
import numpy as np
import ml_dtypes

for _p in ("/opt/trn_rl_repo",):
    if _p not in sys.path:
        sys.path.append(_p)

import concourse.bass as bass
import concourse.mybir as mybir
import concourse.tile as tile
from concourse import bacc
from concourse.bass_utils import run_bass_kernel_spmd

F32 = mybir.dt.float32

B, S, E, H, D = 8, 1500, 1024, 16, 64
P = 128
NEG = -1e9


def _chunks(total, step):
    return [(c0, min(step, total - c0)) for c0 in range(0, total, step)]


def _wslices(dram_ap, col0, cols):
    """[E, E] weight -> [P, E//P, cols] AP for a column slice (k on partition)."""
    return dram_ap.rearrange("(kt p) f -> p kt f", p=P)[:, :, col0:col0 + cols]


def build(causal: bool, mm_dt=mybir.dt.bfloat16):
    KT_N = E // P            # k-tiles over the embedding dim
    FT_N = E // P            # f-tiles
    R_CH = _chunks(S, 512)   # i/r chunks
    JB = _chunks(S, P)       # j blocks
    H_LOC = E // D
    nc = bacc.Bacc("TRN2", target_bir_lowering=False, debug=False, num_devices=8)
    MD = mm_dt  # dtype for every matmul operand chain
    NRM = mybir.dt.float32r if MD == mybir.dt.bfloat16 else MD

    xT = nc.dram_tensor("xT", [E, S], MD, kind="ExternalInput")
    wqT = nc.dram_tensor("wqT", [E, E], MD, kind="ExternalInput")
    wkT = nc.dram_tensor("wkT", [E, E], MD, kind="ExternalInput")
    wvT = nc.dram_tensor("wvT", [E, E], MD, kind="ExternalInput")
    woT = nc.dram_tensor("woT", [E, E], MD, kind="ExternalInput")
    bq = nc.dram_tensor("bq", [E], F32, kind="ExternalInput")
    bo = nc.dram_tensor("bo", [E], F32, kind="ExternalInput")
    maskT = None
    if not causal:
        maskT = nc.dram_tensor("maskT", [S, S], F32, kind="ExternalInput")
    yT = nc.dram_tensor("yT", [E, S], F32, kind="ExternalOutput")

    def mm(ap):
        return ap

    nc._allow_low_precision_reason = "low-precision matmul operand chain"
    with tile.TileContext(nc) as tc:
        with (
            tc.tile_pool(name="persist", bufs=1) as pers,
            tc.tile_pool(name="wqkp", bufs=2) as wqkp,
            tc.tile_pool(name="wvp", bufs=1) as wvp,
            tc.tile_pool(name="wop", bufs=1) as wop,
            tc.tile_pool(name="attn", bufs=3) as apool,
            tc.tile_pool(name="small", bufs=3) as spool,
            tc.tile_pool(name="evp", bufs=3) as evp,
            tc.tile_pool(name="psP", bufs=1, space="PSUM") as psP,
            tc.tile_pool(name="psS", bufs=1, space="PSUM") as psS,
            tc.tile_pool(name="psO", bufs=1, space="PSUM") as psO,
            tc.tile_pool(name="psY", bufs=1, space="PSUM") as psY,
        ):
            ones64 = pers.tile([1, D], NRM, name="ones64")
            nc.vector.memset(ones64[:].bitcast(F32), 1.0)
            bq_sb = pers.tile([P, FT_N], F32, name="bq_sb")
            nc.sync.dma_start(out=bq_sb[:], in_=bq.ap().rearrange("(t p) -> p t", p=P))
            bo_sb = pers.tile([P, FT_N], F32, name="bo_sb")
            nc.sync.dma_start(out=bo_sb[:], in_=bo.ap().rearrange("(t p) -> p t", p=P))

            # upper-triangular (incl diag) 0/1 mask for diagonal attn blocks
            tri32 = pers.tile([P, P], F32, name="tri32")
            nc.gpsimd.memset(tri32[:], 1.0)
            nc.gpsimd.affine_select(
                out=tri32[:], in_=tri32[:],
                pattern=[[1, P]], compare_op=mybir.AluOpType.is_ge,
                fill=0.0, base=0, channel_multiplier=-1,
            )
            tri = pers.tile([P, P], MD, name="tri")
            nc.vector.tensor_copy(out=tri[:], in_=tri32[:])

            XT = [pers.tile([P, S], MD, name=f"xt{kt}") for kt in range(KT_N)]
            QT = [pers.tile([P, S], MD, name=f"qt{ft}") for ft in range(FT_N)]
            KTs = [pers.tile([P, S], MD, name=f"kt{ft}") for ft in range(FT_N)]
            VA = [pers.tile([P, H_LOC * (D + 1)], MD, name=f"va{rt}")
                  for rt in range(len(JB))]
            AOT = [pers.tile([P, S], MD, name=f"aot{ft}") for ft in range(FT_N)]

            # ---- V projection (natural layout, ones column appended) ----
            hpc = 512 // D  # heads per 512-wide f chunk
            fchunks = _chunks(E, 512)
            wv_tiles = [wvp.tile([P, KT_N, 512], MD, name=f"wv{fc}", tag=f"wv{fc}")
                        for fc in range(len(fchunks))]
            # interleave wv-slice and xT-tile loads so the first V matmuls
            # can issue as soon as (wv[:,0,:], xT[0]) land
            for kt in range(KT_N):
                for fc, (f0, fw) in enumerate(fchunks):
                    nc.sync.dma_start(
                        out=wv_tiles[fc][:, kt, :fw],
                        in_=_wslices(wvT.ap(), f0, fw)[:, kt, :])
                nc.sync.dma_start(out=XT[kt][:],
                                  in_=xT[kt * P:(kt + 1) * P, :])

            def emit_v(rts):
                for rt in rts:
                    if rt >= len(JB):
                        continue
                    r0, rsz = JB[rt]
                    for fc, (f0, fw) in enumerate(fchunks):
                        wt = wv_tiles[fc]
                        ps = psP.tile([P, 512], F32, name="pv", tag="pp", bufs=2)
                        for kt in range(KT_N):
                            nc.tensor.matmul(
                                ps[:rsz, :fw],
                                mm(XT[kt][:, r0:r0 + rsz]),
                                mm(wt[:, kt, :fw]),
                                start=(kt == 0), stop=(kt == KT_N - 1),
                            )
                        dst = VA[rt][:].rearrange("p (h c) -> p h c", c=D + 1)
                        nc.vector.tensor_copy(
                            out=dst[:rsz, fc * hpc:fc * hpc + fw // D, 0:D],
                            in_=ps[:rsz, :fw].rearrange("p (h d) -> p h d", d=D),
                        )
                    va3 = VA[rt][:].rearrange("p (h c) -> p h c", c=D + 1)
                    if MD == mybir.dt.float32r:
                        nc.gpsimd.memset(va3[:rsz, :, D:D + 1].bitcast(F32), 1.0)
                    else:
                        nc.gpsimd.memset(va3[:rsz, :, D:D + 1], 1.0)

            def proj_qk_gen(ft):
                for which, wdram, dst in (("q", wqT, QT), ("k", wkT, KTs)):
                    wt = wqkp.tile([P, KT_N, P], MD, name="wqk", tag="wqk")
                    nc.sync.dma_start(out=wt[:], in_=_wslices(wdram.ap(), ft * P, P))
                    for rc, (c0, cw) in enumerate(R_CH):
                        ps = psP.tile([P, 512], F32, name="pp", tag="pp", bufs=2)
                        for kt in range(KT_N):
                            nc.tensor.matmul(
                                ps[:, :cw],
                                mm(wt[:, kt, :]),
                                mm(XT[kt][:, c0:c0 + cw]),
                                start=(kt == 0), stop=(kt == KT_N - 1),
                            )
                        if which == "q":
                            nc.vector.tensor_scalar(
                                out=dst[ft][:, c0:c0 + cw], in0=ps[:, :cw],
                                scalar1=bq_sb[:, ft:ft + 1], scalar2=None,
                                op0=mybir.AluOpType.add,
                            )
                        else:
                            nc.vector.tensor_copy(
                                out=dst[ft][:, c0:c0 + cw], in_=ps[:, :cw])
                        yield

            def proj_qk(ft):
                for _ in proj_qk_gen(ft):
                    pass

            def attn_ft(ic, ft, mtiles, filler=None):
                c0, cw = R_CH[ic]
                nblk = (min(len(JB), (c0 + cw + P - 1) // P)
                        if causal else len(JB))
                pso = [psO.tile([D + 1, 512], F32, name=f"po{half}",
                                tag="po", bufs=2)
                       for half in range(2)]
                # diagonal-containing blocks first so the chunk-end attnV
                # gates on a short (non-masked) exp chain
                if causal:
                    cut = max(0, nblk - (cw + P - 1) // P)
                    order = list(range(cut, nblk)) + list(range(cut))
                else:
                    order = list(range(nblk))
                for n_i, jb in enumerate(order):
                    j0, jsz = JB[jb]
                    vo = max(0, j0 - c0) if causal else 0
                    # both halves' scores land in one 2-bank psum pair so a
                    # single ACTIVATE exps them together (halves ACT op count)
                    psp = psS.tile([P, 2, 512], F32, name="psp",
                                   tag="ps", bufs=2)
                    for half in range(2):
                        d0 = D * half
                        nc.tensor.matmul(
                            psp[:jsz, half, vo:cw],
                            mm(KTs[ft][d0:d0 + D, j0:j0 + jsz]),
                            mm(QT[ft][d0:d0 + D, c0 + vo:c0 + cw]),
                            start=True, stop=True,
                            tile_position=(d0, 0),
                        )
                    if not causal:
                        for half in range(2):
                            nc.vector.tensor_tensor(
                                out=psp[:jsz, half, :cw],
                                in0=psp[:jsz, half, :cw],
                                in1=mtiles[jb][:jsz, :cw],
                                op=mybir.AluOpType.add,
                            )
                    atp = apool.tile([P, 2, 512], MD, name="atp")
                    nc.scalar.activation(
                        out=atp[:jsz, :, vo:cw], in_=psp[:jsz, :, vo:cw],
                        func=mybir.ActivationFunctionType.Exp,
                    )
                    if causal and j0 >= c0:
                        # zero attn where j > i on the diagonal square
                        for half in range(2):
                            nc.vector.tensor_tensor(
                                out=atp[:jsz, half, vo:vo + jsz],
                                in0=atp[:jsz, half, vo:vo + jsz],
                                in1=tri[:jsz, :jsz],
                                op=mybir.AluOpType.mult,
                            )
                    va3 = VA[jb][:].rearrange("p (h c) -> p h c", c=D + 1)
                    for half in range(2):
                        nc.tensor.matmul(
                            pso[half][:, vo:cw],
                            mm(va3[:jsz, 2 * ft + half, :]),
                            mm(atp[:jsz, half, vo:cw]),
                            start=(n_i == 0), stop=(n_i == nblk - 1),
                        )
                    if filler is not None and n_i % 4 == 3:
                        filler()
                ssums = []
                for half in range(2):
                    ssum = spool.tile([1, 512], NRM, name=f"ssum{half}",
                                      tag="ssum")
                    nc.vector.tensor_copy(
                        out=ssum[:, :cw], in_=pso[half][D:D + 1, :cw])
                    ssums.append(ssum)
                for half in range(2):
                    d0 = D * half
                    psb = psP.tile([D, 512], F32, name="psb", tag="pp", bufs=2)
                    nc.tensor.matmul(
                        psb[:, :cw], mm(ones64[:, :]), mm(ssums[half][:, :cw]),
                        start=True, stop=True,
                    )
                    rb = spool.tile([D, 512], F32, name="rb")
                    nc.vector.reciprocal_approx_fast(
                        out=rb[:, :cw], in_=psb[:, :cw])
                    nc.vector.tensor_tensor(
                        out=AOT[ft][d0:d0 + D, c0:c0 + cw],
                        in0=pso[half][0:D, :cw], in1=rb[:, :cw],
                        op=mybir.AluOpType.mult,
                    )

            def emit_yt(ot, rc, wo_t):
                c0, cw = R_CH[rc]
                psy = psP.tile([P, 512], F32, name="py", tag="pp", bufs=2)
                for ft in range(FT_N):
                    nc.tensor.matmul(
                        psy[:, :cw],
                        mm(wo_t[:, ft, :]),
                        mm(AOT[ft][:, c0:c0 + cw]),
                        start=(ft == 0), stop=(ft == FT_N - 1),
                    )
                yt = evp.tile([P, 512], F32, name="yt", tag="yt")
                nc.vector.tensor_scalar(
                    out=yt[:, :cw], in0=psy[:, :cw],
                    scalar1=bo_sb[:, ot:ot + 1], scalar2=None,
                    op0=mybir.AluOpType.add,
                )
                nc.sync.dma_start(
                    out=yT[ot * P:(ot + 1) * P, c0:c0 + cw], in_=yt[:, :cw])

            if causal:
                wo_tiles = []
                for ot in range(FT_N):
                    wt = wop.tile([P, KT_N, P], MD, name=f"wo{ot}",
                                  tag=f"wo{ot}")
                    nc.sync.dma_start(out=wt[:],
                                      in_=_wslices(woT.ap(), ot * P, P))
                    wo_tiles.append(wt)
                nb0 = min(len(JB), (R_CH[0][0] + R_CH[0][1] + P - 1) // P)
                emit_v(range(nb0))
                proj_qk(0)
                nbp = nb0
                for ft in range(FT_N):
                    gen = proj_qk_gen(ft + 1) if ft + 1 < FT_N else None

                    def pump():
                        if gen is not None:
                            next(gen, None)

                    for ic in range(len(R_CH)):
                        attn_ft(ic, ft, None, filler=pump)
                        if ft == 0 and ic + 1 < len(R_CH):
                            c0n, cwn = R_CH[ic + 1]
                            nbn = min(len(JB), (c0n + cwn + P - 1) // P)
                            emit_v(range(nbp, nbn))
                            nbp = nbn
                        if ft == FT_N - 1:
                            # last ft has no next-ft projection filler: use the
                            # now-ready yT chunk as PE filler instead
                            for ot in range(FT_N):
                                emit_yt(ot, ic, wo_tiles[ot])
                    if gen is not None:
                        for _ in gen:
                            pass
            else:
                emit_v(range(len(JB)))
                for ft in range(FT_N):
                    proj_qk(ft)
                with tc.tile_pool(name="maskp", bufs=1) as mpool:
                    for ic, (c0, cw) in enumerate(R_CH):
                        mtiles = []
                        for jb, (j0, jsz) in enumerate(JB):
                            mt = mpool.tile([P, 512], F32, name=f"m{jb}")
                            nc.sync.dma_start(
                                out=mt[:jsz, :cw],
                                in_=maskT[j0:j0 + jsz, c0:c0 + cw])
                            mtiles.append(mt)
                        for ft in range(FT_N):
                            attn_ft(ic, ft, mtiles)
                for ot in range(FT_N):
                    wt = wop.tile([P, KT_N, P], MD, name=f"wo{ot}", tag="wo",
                                  bufs=2)
                    nc.sync.dma_start(out=wt[:], in_=_wslices(woT.ap(), ot * P, P))
                    for rc in range(len(R_CH)):
                        emit_yt(ot, rc, wt)

    nc.compile()
    return nc


def build2(score_fp8: bool = True, mm_dt=mybir.dt.bfloat16):
    """Causal-only v2: gap-free PE schedule + optional fp8 DoubleRow scores.

    Emission-order discipline: the PE queue is in-order, so every
    potentially-waiting matmul has independent filler work (next-ft Q/K
    projection, V-projection, or output-projection matmuls) queued ahead
    of it.  Fillers are drained from a FIFO of generators, with a burst
    at every (ft, chunk) boundary so the softmax-normalize tail and the
    next chunk's first exp are fully hidden.  PE gaps also reset the PE
    clock ramp (0.65/1.2/2.4 GHz p-states), so continuity matters twice.

    Normalize uses one K=2 rank-2 matmul per (ft, chunk) broadcasting
    both halves' row-sums (f32r), with the reciprocal on a [128, cw]
    tile.  Diagonal-block causal masking runs on the idle Pool engine
    via affine_select; row-sum copies also go to Pool.

    score_fp8: Q/K are evicted to fp8e4 in DoubleRow-packed layout
    ([32 partitions, 2 k-planes]) and scores run as fp8 DoubleRow
    matmuls at 0.5 cycles/column.
    """
    KT_N = E // P
    FT_N = E // P
    R_CH = _chunks(S, 512)
    JB = _chunks(S, P)
    H_LOC = E // D
    nc = bacc.Bacc("TRN2", target_bir_lowering=False, debug=False, num_devices=8)
    MD = mm_dt
    F8 = mybir.dt.float8e4
    NRM = mybir.dt.float32r

    xT = nc.dram_tensor("xT", [E, S], MD, kind="ExternalInput")
    wqT = nc.dram_tensor("wqT", [E, E], MD, kind="ExternalInput")
    wkT = nc.dram_tensor("wkT", [E, E], MD, kind="ExternalInput")
    wvT = nc.dram_tensor("wvT", [E, E], MD, kind="ExternalInput")
    woT = nc.dram_tensor("woT", [E, E], MD, kind="ExternalInput")
    bq = nc.dram_tensor("bq", [E], F32, kind="ExternalInput")
    bo = nc.dram_tensor("bo", [E], F32, kind="ExternalInput")
    yT = nc.dram_tensor("yT", [E, S], F32, kind="ExternalOutput")

    nc._allow_low_precision_reason = "low-precision matmul operand chain"
    with tile.TileContext(nc) as tc:
        with (
            tc.tile_pool(name="persist", bufs=1) as pers,
            tc.tile_pool(name="wpool", bufs=1) as wpool,
            tc.tile_pool(name="attn", bufs=3) as apool,
            tc.tile_pool(name="small", bufs=2) as spool,
            tc.tile_pool(name="evp", bufs=3) as evp,
            tc.tile_pool(name="psP", bufs=1, space="PSUM") as psP,
            tc.tile_pool(name="psS", bufs=1, space="PSUM") as psS,
            tc.tile_pool(name="psO", bufs=1, space="PSUM") as psO,
        ):
            # ---- persistent small tiles / DMA preload (priority order) ----
            bq_sb = pers.tile([P, FT_N], F32, name="bq_sb")
            nc.sync.dma_start(out=bq_sb[:], in_=bq.ap().rearrange("(t p) -> p t", p=P))
            bo_sb = pers.tile([P, FT_N], F32, name="bo_sb")
            nc.sync.dma_start(out=bo_sb[:], in_=bo.ap().rearrange("(t p) -> p t", p=P))

            # Q/K weights streamed per-ft (4 rotating slots, DMA issued
            # ~2 chunks ahead of first use)
            wqk_t: dict = {}

            def dma_wqk(ft):
                if ft in wqk_t or ft >= FT_N:
                    return
                tq = wpool.tile([P, KT_N, P], MD, name=f"wq{ft}", tag="wqk",
                                bufs=4)
                nc.sync.dma_start(out=tq[:], in_=_wslices(wqT.ap(), ft * P, P))
                tk = wpool.tile([P, KT_N, P], MD, name=f"wk{ft}", tag="wqk",
                                bufs=4)
                nc.sync.dma_start(out=tk[:], in_=_wslices(wkT.ap(), ft * P, P))
                wqk_t[ft] = (tq, tk)

            dma_wqk(0)

            # xT and wv interleaved per-kt (startup critical path)
            XT = [pers.tile([P, S], MD, name=f"xt{kt}") for kt in range(KT_N)]
            fchunks = _chunks(E, 512)
            hpc = 512 // D
            wv_tiles = [wpool.tile([P, KT_N, 512], MD, name=f"wv{fc}")
                        for fc in range(len(fchunks))]
            for kt in range(KT_N):
                nc.sync.dma_start(out=XT[kt][:], in_=xT[kt * P:(kt + 1) * P, :])
                for fc, (f0, fw) in enumerate(fchunks):
                    nc.sync.dma_start(
                        out=wv_tiles[fc][:, kt, :fw],
                        in_=_wslices(wvT.ap(), f0, fw)[:, kt, :])
            dma_wqk(1)
            # output-projection weights last
            wo_tiles = [wpool.tile([P, KT_N, P], MD, name=f"wo{ot}")
                        for ot in range(FT_N)]
            for ot in range(FT_N):
                nc.sync.dma_start(out=wo_tiles[ot][:], in_=_wslices(woT.ap(), ot * P, P))

            # sel2: [2, 128], row0 = 1 on cols 0-63, row1 = 1 on cols 64-127
            sel2f = pers.tile([2, P], F32, name="sel2f")
            nc.gpsimd.memset(sel2f[:], 1.0)
            nc.gpsimd.affine_select(
                out=sel2f[:], in_=sel2f[:],
                pattern=[[1, P]], compare_op=mybir.AluOpType.is_ge,
                fill=0.0, base=0, channel_multiplier=-D,
            )
            nc.gpsimd.affine_select(
                out=sel2f[:], in_=sel2f[:],
                pattern=[[-1, P]], compare_op=mybir.AluOpType.is_ge,
                fill=0.0, base=D - 1, channel_multiplier=D,
            )
            sel2 = pers.tile([2, P], NRM, name="sel2")
            nc.vector.tensor_copy(out=sel2[:], in_=sel2f[:])

            VA = [pers.tile([P, H_LOC * (D + 1)], MD, name=f"va{rt}")
                  for rt in range(len(JB))]
            AOT = [pers.tile([P, S], MD, name=f"aot{ft}") for ft in range(FT_N)]
            if score_fp8:
                # DoubleRow-packed Q/K per ft, fp8.  Partitions [32h:32h+32)
                # hold head h's 64 dims as 2 k-planes (d = 32*plane + p).
                # Q is the moving operand (no stride restriction): [2, S].
                # K is the stationary operand — LDWEIGHTS dual-fp8 requires a
                # power-of-two plane stride, so K is blocked per j-block of
                # 128 with padding: [12, 2, 128].
                Q8 = [pers.tile([P, 2, S], F8, name=f"q8_{ft}")
                      for ft in range(FT_N)]
                K8 = [pers.tile([P, len(JB), 2, P], F8, name=f"k8_{ft}")
                      for ft in range(FT_N)]
            else:
                QT = [pers.tile([P, S], MD, name=f"qt{ft}") for ft in range(FT_N)]
                KTs = [pers.tile([P, S], MD, name=f"kt{ft}") for ft in range(FT_N)]

            # ---------------- work generators (PE-quantum yields) --------
            v_done = [0]  # highest V block index fully emitted + 1

            def gen_v(rts):
                for rt in rts:
                    r0, rsz = JB[rt]
                    for fc, (f0, fw) in enumerate(fchunks):
                        ps = psP.tile([P, 512], F32, name="pv", tag="pp", bufs=2)
                        for kt in range(KT_N):
                            nc.tensor.matmul(
                                ps[:rsz, :fw],
                                XT[kt][:, r0:r0 + rsz],
                                wv_tiles[fc][:, kt, :fw],
                                start=(kt == 0), stop=(kt == KT_N - 1),
                            )
                            yield
                        dst = VA[rt][:].rearrange("p (h c) -> p h c", c=D + 1)
                        nc.vector.tensor_copy(
                            out=dst[:rsz, fc * hpc:fc * hpc + fw // D, 0:D],
                            in_=ps[:rsz, :fw].rearrange("p (h d) -> p h d", d=D),
                        )
                    va3 = VA[rt][:].rearrange("p (h c) -> p h c", c=D + 1)
                    nc.gpsimd.memset(va3[:rsz, :, D:D + 1], 1.0)
                    v_done[0] = rt + 1

            p_done = [0] * FT_N  # chunks (q+k pairs) emitted per ft

            def gen_projqk(ft):
                for rc, (c0, cw) in enumerate(R_CH):
                    for which in range(2):  # 0=q, 1=k
                        wt = wqk_t[ft][which]
                        ps = psP.tile([P, 512], F32, name="pp", tag="pp", bufs=2)
                        for kt in range(KT_N):
                            nc.tensor.matmul(
                                ps[:, :cw],
                                wt[:, kt, :],
                                XT[kt][:, c0:c0 + cw],
                                start=(kt == 0), stop=(kt == KT_N - 1),
                            )
                            yield
                        if score_fp8:
                            stg = spool.tile([P, 512], F8, name="stg", tag="stg",
                                             bufs=2)
                            if which == 0:
                                nc.vector.tensor_scalar(
                                    out=stg[:, :cw], in0=ps[:, :cw],
                                    scalar1=bq_sb[:, ft:ft + 1], scalar2=None,
                                    op0=mybir.AluOpType.add,
                                )
                            else:
                                nc.vector.tensor_copy(out=stg[:, :cw], in_=ps[:, :cw])
                            for h in range(2):
                                for j in range(2):
                                    src = stg[64 * h + 32 * j:
                                              64 * h + 32 * (j + 1), :cw]
                                    if which == 0:
                                        nc.sync.dma_start(
                                            out=Q8[ft][32 * h:32 * (h + 1),
                                                       j, c0:c0 + cw],
                                            in_=src)
                                    else:
                                        # K: into 128-col padded blocks
                                        b0 = 4 * rc
                                        nfull = cw // P
                                        nc.sync.dma_start(
                                            out=K8[ft][32 * h:32 * (h + 1),
                                                       b0:b0 + nfull, j, :],
                                            in_=src[:, :nfull * P].rearrange(
                                                "p (b c) -> p b c", c=P))
                                        if cw % P:
                                            nc.sync.dma_start(
                                                out=K8[ft][32 * h:32 * (h + 1),
                                                           b0 + nfull, j,
                                                           :cw % P],
                                                in_=src[:, nfull * P:cw])
                        else:
                            dst = QT if which == 0 else KTs
                            if which == 0:
                                nc.vector.tensor_scalar(
                                    out=dst[ft][:, c0:c0 + cw], in0=ps[:, :cw],
                                    scalar1=bq_sb[:, ft:ft + 1], scalar2=None,
                                    op0=mybir.AluOpType.add,
                                )
                            else:
                                nc.vector.tensor_copy(
                                    out=dst[ft][:, c0:c0 + cw], in_=ps[:, :cw])
                    p_done[ft] = rc + 1

            def gen_yt(rc):
                c0, cw = R_CH[rc]
                for ot in range(FT_N):
                    psy = psP.tile([P, 512], F32, name="py", tag="pp", bufs=2)
                    for ft in range(FT_N):
                        nc.tensor.matmul(
                            psy[:, :cw],
                            wo_tiles[ot][:, ft, :],
                            AOT[ft][:, c0:c0 + cw],
                            start=(ft == 0), stop=(ft == FT_N - 1),
                        )
                        yield
                    ytt = evp.tile([P, 512], F32, name="yt", tag="yt")
                    nc.vector.tensor_scalar(
                        out=ytt[:, :cw], in0=psy[:, :cw],
                        scalar1=bo_sb[:, ot:ot + 1], scalar2=None,
                        op0=mybir.AluOpType.add,
                    )
                    nc.sync.dma_start(
                        out=yT[ot * P:(ot + 1) * P, c0:c0 + cw], in_=ytt[:, :cw])

            # ---------------- filler FIFO ---------------------------------
            fillers: list = []

            def pump(n=1):
                while n > 0 and fillers:
                    try:
                        next(fillers[0])
                        n -= 1
                    except StopIteration:
                        fillers.pop(0)

            def drain(g):
                if g is None:
                    return
                for _ in g:
                    pass
                if fillers and fillers[0] is g:
                    fillers.pop(0)

            def ensure(g, get, target):
                while get() < target:
                    try:
                        next(g)
                    except StopIteration:
                        break

            # ---------------- attention -----------------------------------
            pending_norm = [None]

            def do_norm():
                fn = pending_norm[0]
                pending_norm[0] = None
                if fn is not None:
                    fn()

            def attn_chunk(ft, ic):
                c0, cw = R_CH[ic]
                nblk = min(len(JB), (c0 + cw + P - 1) // P)
                cut = max(0, nblk - (cw + P - 1) // P)
                order = list(range(cut, nblk)) + list(range(cut))
                pso = [psO.tile([D + 1, 512], F32, name=f"po{h}", tag="po", bufs=2)
                       for h in range(2)]
                for n_i, jb in enumerate(order):
                    j0, jsz = JB[jb]
                    vo = max(0, j0 - c0)
                    cwv = cw - vo
                    psp = psS.tile([P, 2, 512], F32, name="psp", tag="ps", bufs=2)
                    if score_fp8:
                        for h in range(2):
                            nc.tensor.matmul(
                                psp[:jsz, h, vo:cw],
                                K8[ft][32 * h:32 * (h + 1), jb, :, :jsz],
                                Q8[ft][32 * h:32 * (h + 1), :,
                                       c0 + vo:c0 + cw],
                                start=True, stop=True,
                                perf_mode=mybir.MatmulPerfMode.DoubleRow,
                                tile_position=(32 * h, 0),
                            )
                    else:
                        for h in range(2):
                            d0 = D * h
                            nc.tensor.matmul(
                                psp[:jsz, h, vo:cw],
                                KTs[ft][d0:d0 + D, j0:j0 + jsz],
                                QT[ft][d0:d0 + D, c0 + vo:c0 + cw],
                                start=True, stop=True,
                                tile_position=(d0, 0),
                            )
                    if n_i == 0:
                        # boundary: the ensure_* burst was just emitted ahead
                        # of us; a little more filler hides the normalize
                        # tail and this chunk's first exp
                        pump(2)
                        do_norm()
                        pump(6)
                    else:
                        pump(1)
                    atp = apool.tile([P, 2, 512], MD, name="atp")
                    nc.scalar.activation(
                        out=atp[:jsz, :, vo:cw], in_=psp[:jsz, :, vo:cw],
                        func=mybir.ActivationFunctionType.Exp,
                    )
                    if j0 >= c0:
                        # zero attn where j > i on the diagonal square (Pool)
                        nc.gpsimd.affine_select(
                            out=atp[:jsz, :, vo:cw], in_=atp[:jsz, :, vo:cw],
                            pattern=[[0, 2], [1, cwv]],
                            compare_op=mybir.AluOpType.is_ge,
                            fill=0.0, base=0, channel_multiplier=-1,
                        )
                    va3 = VA[jb][:].rearrange("p (h c) -> p h c", c=D + 1)
                    for h in range(2):
                        nc.tensor.matmul(
                            pso[h][:, vo:cw],
                            va3[:jsz, 2 * ft + h, :],
                            atp[:jsz, h, vo:cw],
                            start=(n_i == 0), stop=(n_i == nblk - 1),
                        )
                # row-sums: DVE can only write partition 0, so both halves'
                # rows land flat on partition 0, then one SBUF->SBUF DMA
                # spreads them across partitions 0-1 for the K=2 broadcast
                # matmul; the rest of the normalize is deferred
                ssumf = spool.tile([1, 2, 512], NRM, name="ssumf", tag="ssumf")
                for h in range(2):
                    nc.vector.tensor_copy(
                        out=ssumf[0:1, h, :cw], in_=pso[h][D:D + 1, :cw])
                ssum2 = spool.tile([2, 512], NRM, name="ssum2", tag="ssum2")
                nc.sync.dma_start(out=ssum2[0:2, :cw], in_=ssumf[0:1, :, :cw])

                def norm(ft=ft, ic=ic, pso=pso, ssum2=ssum2, c0=c0, cw=cw):
                    psb = psP.tile([P, 512], F32, name="psb", tag="pp", bufs=2)
                    nc.tensor.matmul(
                        psb[:, :cw], sel2[:, :], ssum2[:, :cw],
                        start=True, stop=True,
                    )
                    rb = spool.tile([P, 512], F32, name="rb", tag="rb", bufs=2)
                    nc.vector.reciprocal_approx_fast(out=rb[:, :cw], in_=psb[:, :cw])
                    for h in range(2):
                        nc.vector.tensor_tensor(
                            out=AOT[ft][D * h:D * (h + 1), c0:c0 + cw],
                            in0=pso[h][0:D, :cw], in1=rb[D * h:D * (h + 1), :cw],
                            op=mybir.AluOpType.mult,
                        )
                pending_norm[0] = norm

            # ---------------- main schedule -------------------------------
            # Startup: V blocks for chunk 0, then Q/K for ft 0 (their
            # matmuls wait on the per-kt xT DMAs, pipelining the load).
            nb0 = min(len(JB), (R_CH[0][0] + R_CH[0][1] + P - 1) // P)
            vgen = gen_v(range(len(JB)))
            ensure(vgen, lambda: v_done[0], nb0)
            projgens = [gen_projqk(ftn) for ftn in range(FT_N)]
            drain(projgens[0])
            fillers.append(vgen)
            for ftn in range(1, FT_N):
                fillers.append(projgens[ftn])

            for ft in range(FT_N):
                for ic in range(len(R_CH)):
                    # demand-driven bursts: everything this chunk's attn
                    # reads must be emitted before it, and the burst doubles
                    # as the boundary filler hiding the normalize tail
                    if ft == 0:
                        c0n, cwn = R_CH[ic]
                        nbn = min(len(JB), (c0n + cwn + P - 1) // P)
                        ensure(vgen, lambda: v_done[0], nbn)
                    if ic == 0 and 1 <= ft:
                        dma_wqk(ft + 1)
                    if ft + 1 < FT_N and ic == len(R_CH) - 1:
                        # pre-emit next ft's first proj chunk before our
                        # last chunk so the ft boundary has cover too
                        ensure(projgens[ft + 1], lambda: p_done[ft + 1], 1)
                    if ft > 0:
                        ensure(projgens[ft], lambda: p_done[ft],
                               min(ic + 2, len(R_CH)))
                    attn_chunk(ft, ic)
                    if ft == FT_N - 1:
                        # yt(ic)'s first 7 accumulations per ot only touch
                        # AOT[0..6] — pre-emit a few as cover, then the
                        # normalize, then the rest follows behind it
                        fillers.append(gen_yt(ic))
                        pump(7)
                        do_norm()
            while fillers:
                pump(10000)

    nc.compile()
    return nc


_CACHE: dict = {}


def _get_nc(causal: bool):
    if causal not in _CACHE:
        _CACHE[causal] = build2() if causal else build(causal)
    return _CACHE[causal]


def _is_causal(mask: np.ndarray) -> bool:
    if mask.shape != (S, S):
        return False
    expect = np.where(np.tril(np.ones((S, S), dtype=bool)), np.float32(0.0),
                      np.float32(NEG))
    return bool(np.array_equal(mask, expect))


MM_NP = ml_dtypes.bfloat16  # numpy dtype matching build()'s default mm_dt


def prep_inputs(x, mask, Wq, bq, Wk, Wv, bv, Wo, bo):
    """Host-side preprocessing shared by kernel() and the bench harness."""
    scale = np.float32(1.0 / np.sqrt(D))
    xT = np.ascontiguousarray(np.transpose(x, (0, 2, 1)).astype(np.float32)).astype(MM_NP)
    common = {
        "wqT": np.ascontiguousarray((Wq.astype(np.float32) * scale).T).astype(MM_NP),
        "wkT": np.ascontiguousarray(Wk.astype(np.float32).T).astype(MM_NP),
        "wvT": np.ascontiguousarray(Wv.astype(np.float32).T).astype(MM_NP),
        "woT": np.ascontiguousarray(Wo.astype(np.float32).T).astype(MM_NP),
        "bq": (bq.astype(np.float32) * scale),
        "bo": (bo.astype(np.float32) + Wo.astype(np.float32) @ bv.astype(np.float32)),
    }
    causal = _is_causal(np.asarray(mask))
    if not causal:
        common["maskT"] = np.ascontiguousarray(np.asarray(mask, np.float32).T)
    in_maps = [dict(common, xT=xT[b]) for b in range(B)]
    return causal, in_maps


_RUNNER: dict = {}


def _get_runner(causal: bool):
    """Compile once per mask-variant; cache the jitted SPMD executable."""
    if causal in _RUNNER:
        return _RUNNER[causal]
    import jax
    from jax.sharding import Mesh, PartitionSpec, NamedSharding
    import warnings
    with warnings.catch_warnings():
        warnings.simplefilter("ignore")
        from jax.experimental.shard_map import shard_map
    from concourse import bass2jax
    from concourse.bass2jax import _bass_exec_p, install_neuronx_cc_hook

    nc = _get_nc(causal)
    install_neuronx_cc_hook()
    partition_name = (nc.partition_id_tensor.name
                      if nc.partition_id_tensor else None)
    in_names, out_names, out_avals = [], [], []
    for alloc in nc.m.functions[0].allocations:
        if not isinstance(alloc, mybir.MemoryLocationSet):
            continue
        name = alloc.memorylocations[0].name
        if alloc.kind == "ExternalInput":
            if name != partition_name:
                in_names.append(name)
        elif alloc.kind == "ExternalOutput":
            out_names.append(name)
            out_avals.append(jax.core.ShapedArray(
                tuple(alloc.tensor_shape), mybir.dt.np(alloc.dtype)))
    n_params = len(in_names)
    n_outs = len(out_names)

    def _body(*args):
        operands = list(args)
        names = list(in_names) + list(out_names)
        if partition_name is not None:
            operands.append(bass2jax.partition_id_tensor())
            names.append(partition_name)
        outs = _bass_exec_p.bind(
            *operands,
            out_avals=tuple(out_avals),
            in_names=tuple(names),
            out_names=tuple(out_names),
            lowering_input_output_aliases=(),
            sim_require_finite=True,
            sim_require_nnan=True,
            nc=nc,
        )
        return tuple(outs)

    devices = jax.devices()[:B]
    mesh = Mesh(np.asarray(devices), ("core",))
    in_specs = (PartitionSpec("core"),) * (n_params + n_outs)
    out_specs = (PartitionSpec("core"),) * n_outs
    fn = jax.jit(
        shard_map(_body, mesh=mesh, in_specs=in_specs, out_specs=out_specs,
                  check_rep=False),
        donate_argnums=tuple(range(n_params, n_params + n_outs)),
        keep_unused=True,
    )
    runner = (fn, in_names, out_names, out_avals)
    _RUNNER[causal] = runner
    return runner


def kernel(x, mask, Wq, bq, Wk, Wv, bv, Wo, bo):
    causal, in_maps = prep_inputs(x, mask, Wq, bq, Wk, Wv, bv, Wo, bo)
    fn, in_names, out_names, out_avals = _get_runner(causal)
    cat = [np.concatenate([np.asarray(m[n]) for m in in_maps], axis=0)
           for n in in_names]
    zs = [np.zeros((B * a.shape[0], *a.shape[1:]), a.dtype) for a in out_avals]
    outs = fn(*cat, *zs)
    yT = np.asarray(outs[out_names.index("yT")]).reshape(B, E, S)
    out = np.ascontiguousarray(yT.transpose(0, 2, 1).astype(np.float32))
    return out



# revision 40
# speedup vs baseline: 1.0304x; 1.0304x over previous
"""Multi-head attention (B=8, S=1500, E=1024, H=16, D=64) on 8 trn2 NeuronCores.

Sharding: pure data-parallel over batch — core b computes batch element b
end-to-end (no collectives). Host pre-transposes x and the weights so every
device-side matmul has its contraction dim on the SBUF partition axis, and
folds the 1/sqrt(D) scale into Wq/bq and the V-bias into the output bias
(bo_eff = bo + Wo @ bv), so the device kernel never touches bv.

Device pipeline per core (all f32 storage, matmuls run as float32r):
  QT = (Wq/8)^T-proj of x^T   [1024, 1500]  (f-on-partition; bias bq/8 per-partition)
  KT = Wk^T-proj              [1024, 1500]
  V_aug = x @ Wv^T with a ones-column appended per head  [1500, 16*65]
  per (i-chunk, head): scoresT[j, i] via matmul(lhsT=KT_h, rhs=QT_h);
    exp on ACT (no max-subtraction: |scores| <~ 30, safely inside f32);
    causal masking structurally (affine_select zero-fill on diagonal blocks)
    or via an additive mask tensor (general path);
  out^T + rowsums in ONE matmul: lhsT=[V_h | 1] (65 cols), rhs=attnT;
  normalize: recip of the sums row, rank-1 matmul broadcast across 64
    partitions, multiply on eviction -> AO^T;
  yT = Wo^T-proj of AO^T + bo_eff  -> DRAM [1024, 1500], host transposes back.
"""

import sys
import numpy as np
import ml_dtypes

for _p in ("/opt/trn_rl_repo",):
    if _p not in sys.path:
        sys.path.append(_p)

import concourse.bass as bass
import concourse.mybir as mybir
import concourse.tile as tile
from concourse import bacc
from concourse.bass_utils import run_bass_kernel_spmd

F32 = mybir.dt.float32

B, S, E, H, D = 8, 1500, 1024, 16, 64
P = 128
NEG = -1e9


def _chunks(total, step):
    return [(c0, min(step, total - c0)) for c0 in range(0, total, step)]


def _wslices(dram_ap, col0, cols):
    """[E, E] weight -> [P, E//P, cols] AP for a column slice (k on partition)."""
    return dram_ap.rearrange("(kt p) f -> p kt f", p=P)[:, :, col0:col0 + cols]


def build(causal: bool, mm_dt=mybir.dt.bfloat16):
    KT_N = E // P            # k-tiles over the embedding dim
    FT_N = E // P            # f-tiles
    R_CH = _chunks(S, 512)   # i/r chunks
    JB = _chunks(S, P)       # j blocks
    H_LOC = E // D
    nc = bacc.Bacc("TRN2", target_bir_lowering=False, debug=False, num_devices=8)
    MD = mm_dt  # dtype for every matmul operand chain
    NRM = mybir.dt.float32r if MD == mybir.dt.bfloat16 else MD

    xT = nc.dram_tensor("xT", [E, S], MD, kind="ExternalInput")
    wqT = nc.dram_tensor("wqT", [E, E], MD, kind="ExternalInput")
    wkT = nc.dram_tensor("wkT", [E, E], MD, kind="ExternalInput")
    wvT = nc.dram_tensor("wvT", [E, E], MD, kind="ExternalInput")
    woT = nc.dram_tensor("woT", [E, E], MD, kind="ExternalInput")
    bq = nc.dram_tensor("bq", [E], F32, kind="ExternalInput")
    bo = nc.dram_tensor("bo", [E], F32, kind="ExternalInput")
    maskT = None
    if not causal:
        maskT = nc.dram_tensor("maskT", [S, S], F32, kind="ExternalInput")
    yT = nc.dram_tensor("yT", [E, S], F32, kind="ExternalOutput")

    def mm(ap):
        return ap

    nc._allow_low_precision_reason = "low-precision matmul operand chain"
    with tile.TileContext(nc) as tc:
        with (
            tc.tile_pool(name="persist", bufs=1) as pers,
            tc.tile_pool(name="wqkp", bufs=2) as wqkp,
            tc.tile_pool(name="wvp", bufs=1) as wvp,
            tc.tile_pool(name="wop", bufs=1) as wop,
            tc.tile_pool(name="attn", bufs=3) as apool,
            tc.tile_pool(name="small", bufs=3) as spool,
            tc.tile_pool(name="evp", bufs=3) as evp,
            tc.tile_pool(name="psP", bufs=1, space="PSUM") as psP,
            tc.tile_pool(name="psS", bufs=1, space="PSUM") as psS,
            tc.tile_pool(name="psO", bufs=1, space="PSUM") as psO,
            tc.tile_pool(name="psY", bufs=1, space="PSUM") as psY,
        ):
            ones64 = pers.tile([1, D], NRM, name="ones64")
            nc.vector.memset(ones64[:].bitcast(F32), 1.0)
            bq_sb = pers.tile([P, FT_N], F32, name="bq_sb")
            nc.sync.dma_start(out=bq_sb[:], in_=bq.ap().rearrange("(t p) -> p t", p=P))
            bo_sb = pers.tile([P, FT_N], F32, name="bo_sb")
            nc.sync.dma_start(out=bo_sb[:], in_=bo.ap().rearrange("(t p) -> p t", p=P))

            # upper-triangular (incl diag) 0/1 mask for diagonal attn blocks
            tri32 = pers.tile([P, P], F32, name="tri32")
            nc.gpsimd.memset(tri32[:], 1.0)
            nc.gpsimd.affine_select(
                out=tri32[:], in_=tri32[:],
                pattern=[[1, P]], compare_op=mybir.AluOpType.is_ge,
                fill=0.0, base=0, channel_multiplier=-1,
            )
            tri = pers.tile([P, P], MD, name="tri")
            nc.vector.tensor_copy(out=tri[:], in_=tri32[:])

            XT = [pers.tile([P, S], MD, name=f"xt{kt}") for kt in range(KT_N)]
            QT = [pers.tile([P, S], MD, name=f"qt{ft}") for ft in range(FT_N)]
            KTs = [pers.tile([P, S], MD, name=f"kt{ft}") for ft in range(FT_N)]
            VA = [pers.tile([P, H_LOC * (D + 1)], MD, name=f"va{rt}")
                  for rt in range(len(JB))]
            AOT = [pers.tile([P, S], MD, name=f"aot{ft}") for ft in range(FT_N)]

            # ---- V projection (natural layout, ones column appended) ----
            hpc = 512 // D  # heads per 512-wide f chunk
            fchunks = _chunks(E, 512)
            wv_tiles = [wvp.tile([P, KT_N, 512], MD, name=f"wv{fc}", tag=f"wv{fc}")
                        for fc in range(len(fchunks))]
            # interleave wv-slice and xT-tile loads so the first V matmuls
            # can issue as soon as (wv[:,0,:], xT[0]) land
            for kt in range(KT_N):
                for fc, (f0, fw) in enumerate(fchunks):
                    nc.sync.dma_start(
                        out=wv_tiles[fc][:, kt, :fw],
                        in_=_wslices(wvT.ap(), f0, fw)[:, kt, :])
                nc.sync.dma_start(out=XT[kt][:],
                                  in_=xT[kt * P:(kt + 1) * P, :])

            def emit_v(rts):
                for rt in rts:
                    if rt >= len(JB):
                        continue
                    r0, rsz = JB[rt]
                    for fc, (f0, fw) in enumerate(fchunks):
                        wt = wv_tiles[fc]
                        ps = psP.tile([P, 512], F32, name="pv", tag="pp", bufs=2)
                        for kt in range(KT_N):
                            nc.tensor.matmul(
                                ps[:rsz, :fw],
                                mm(XT[kt][:, r0:r0 + rsz]),
                                mm(wt[:, kt, :fw]),
                                start=(kt == 0), stop=(kt == KT_N - 1),
                            )
                        dst = VA[rt][:].rearrange("p (h c) -> p h c", c=D + 1)
                        nc.vector.tensor_copy(
                            out=dst[:rsz, fc * hpc:fc * hpc + fw // D, 0:D],
                            in_=ps[:rsz, :fw].rearrange("p (h d) -> p h d", d=D),
                        )
                    va3 = VA[rt][:].rearrange("p (h c) -> p h c", c=D + 1)
                    if MD == mybir.dt.float32r:
                        nc.gpsimd.memset(va3[:rsz, :, D:D + 1].bitcast(F32), 1.0)
                    else:
                        nc.gpsimd.memset(va3[:rsz, :, D:D + 1], 1.0)

            def proj_qk_gen(ft):
                for which, wdram, dst in (("q", wqT, QT), ("k", wkT, KTs)):
                    wt = wqkp.tile([P, KT_N, P], MD, name="wqk", tag="wqk")
                    nc.sync.dma_start(out=wt[:], in_=_wslices(wdram.ap(), ft * P, P))
                    for rc, (c0, cw) in enumerate(R_CH):
                        ps = psP.tile([P, 512], F32, name="pp", tag="pp", bufs=2)
                        for kt in range(KT_N):
                            nc.tensor.matmul(
                                ps[:, :cw],
                                mm(wt[:, kt, :]),
                                mm(XT[kt][:, c0:c0 + cw]),
                                start=(kt == 0), stop=(kt == KT_N - 1),
                            )
                        if which == "q":
                            nc.vector.tensor_scalar(
                                out=dst[ft][:, c0:c0 + cw], in0=ps[:, :cw],
                                scalar1=bq_sb[:, ft:ft + 1], scalar2=None,
                                op0=mybir.AluOpType.add,
                            )
                        else:
                            nc.vector.tensor_copy(
                                out=dst[ft][:, c0:c0 + cw], in_=ps[:, :cw])
                        yield

            def proj_qk(ft):
                for _ in proj_qk_gen(ft):
                    pass

            def attn_ft(ic, ft, mtiles, filler=None):
                c0, cw = R_CH[ic]
                nblk = (min(len(JB), (c0 + cw + P - 1) // P)
                        if causal else len(JB))
                pso = [psO.tile([D + 1, 512], F32, name=f"po{half}",
                                tag="po", bufs=2)
                       for half in range(2)]
                # diagonal-containing blocks first so the chunk-end attnV
                # gates on a short (non-masked) exp chain
                if causal:
                    cut = max(0, nblk - (cw + P - 1) // P)
                    order = list(range(cut, nblk)) + list(range(cut))
                else:
                    order = list(range(nblk))
                for n_i, jb in enumerate(order):
                    j0, jsz = JB[jb]
                    vo = max(0, j0 - c0) if causal else 0
                    # both halves' scores land in one 2-bank psum pair so a
                    # single ACTIVATE exps them together (halves ACT op count)
                    psp = psS.tile([P, 2, 512], F32, name="psp",
                                   tag="ps", bufs=2)
                    for half in range(2):
                        d0 = D * half
                        nc.tensor.matmul(
                            psp[:jsz, half, vo:cw],
                            mm(KTs[ft][d0:d0 + D, j0:j0 + jsz]),
                            mm(QT[ft][d0:d0 + D, c0 + vo:c0 + cw]),
                            start=True, stop=True,
                            tile_position=(d0, 0),
                        )
                    if not causal:
                        for half in range(2):
                            nc.vector.tensor_tensor(
                                out=psp[:jsz, half, :cw],
                                in0=psp[:jsz, half, :cw],
                                in1=mtiles[jb][:jsz, :cw],
                                op=mybir.AluOpType.add,
                            )
                    atp = apool.tile([P, 2, 512], MD, name="atp")
                    nc.scalar.activation(
                        out=atp[:jsz, :, vo:cw], in_=psp[:jsz, :, vo:cw],
                        func=mybir.ActivationFunctionType.Exp,
                    )
                    if causal and j0 >= c0:
                        # zero attn where j > i on the diagonal square
                        for half in range(2):
                            nc.vector.tensor_tensor(
                                out=atp[:jsz, half, vo:vo + jsz],
                                in0=atp[:jsz, half, vo:vo + jsz],
                                in1=tri[:jsz, :jsz],
                                op=mybir.AluOpType.mult,
                            )
                    va3 = VA[jb][:].rearrange("p (h c) -> p h c", c=D + 1)
                    for half in range(2):
                        nc.tensor.matmul(
                            pso[half][:, vo:cw],
                            mm(va3[:jsz, 2 * ft + half, :]),
                            mm(atp[:jsz, half, vo:cw]),
                            start=(n_i == 0), stop=(n_i == nblk - 1),
                        )
                    if filler is not None and n_i % 4 == 3:
                        filler()
                ssums = []
                for half in range(2):
                    ssum = spool.tile([1, 512], NRM, name=f"ssum{half}",
                                      tag="ssum")
                    nc.vector.tensor_copy(
                        out=ssum[:, :cw], in_=pso[half][D:D + 1, :cw])
                    ssums.append(ssum)
                for half in range(2):
                    d0 = D * half
                    psb = psP.tile([D, 512], F32, name="psb", tag="pp", bufs=2)
                    nc.tensor.matmul(
                        psb[:, :cw], mm(ones64[:, :]), mm(ssums[half][:, :cw]),
                        start=True, stop=True,
                    )
                    rb = spool.tile([D, 512], F32, name="rb")
                    nc.vector.reciprocal_approx_fast(
                        out=rb[:, :cw], in_=psb[:, :cw])
                    nc.vector.tensor_tensor(
                        out=AOT[ft][d0:d0 + D, c0:c0 + cw],
                        in0=pso[half][0:D, :cw], in1=rb[:, :cw],
                        op=mybir.AluOpType.mult,
                    )

            def emit_yt(ot, rc, wo_t):
                c0, cw = R_CH[rc]
                psy = psP.tile([P, 512], F32, name="py", tag="pp", bufs=2)
                for ft in range(FT_N):
                    nc.tensor.matmul(
                        psy[:, :cw],
                        mm(wo_t[:, ft, :]),
                        mm(AOT[ft][:, c0:c0 + cw]),
                        start=(ft == 0), stop=(ft == FT_N - 1),
                    )
                yt = evp.tile([P, 512], F32, name="yt", tag="yt")
                nc.vector.tensor_scalar(
                    out=yt[:, :cw], in0=psy[:, :cw],
                    scalar1=bo_sb[:, ot:ot + 1], scalar2=None,
                    op0=mybir.AluOpType.add,
                )
                nc.sync.dma_start(
                    out=yT[ot * P:(ot + 1) * P, c0:c0 + cw], in_=yt[:, :cw])

            if causal:
                wo_tiles = []
                for ot in range(FT_N):
                    wt = wop.tile([P, KT_N, P], MD, name=f"wo{ot}",
                                  tag=f"wo{ot}")
                    nc.sync.dma_start(out=wt[:],
                                      in_=_wslices(woT.ap(), ot * P, P))
                    wo_tiles.append(wt)
                nb0 = min(len(JB), (R_CH[0][0] + R_CH[0][1] + P - 1) // P)
                emit_v(range(nb0))
                proj_qk(0)
                nbp = nb0
                for ft in range(FT_N):
                    gen = proj_qk_gen(ft + 1) if ft + 1 < FT_N else None

                    def pump():
                        if gen is not None:
                            next(gen, None)

                    for ic in range(len(R_CH)):
                        attn_ft(ic, ft, None, filler=pump)
                        if ft == 0 and ic + 1 < len(R_CH):
                            c0n, cwn = R_CH[ic + 1]
                            nbn = min(len(JB), (c0n + cwn + P - 1) // P)
                            emit_v(range(nbp, nbn))
                            nbp = nbn
                        if ft == FT_N - 1:
                            # last ft has no next-ft projection filler: use the
                            # now-ready yT chunk as PE filler instead
                            for ot in range(FT_N):
                                emit_yt(ot, ic, wo_tiles[ot])
                    if gen is not None:
                        for _ in gen:
                            pass
            else:
                emit_v(range(len(JB)))
                for ft in range(FT_N):
                    proj_qk(ft)
                with tc.tile_pool(name="maskp", bufs=1) as mpool:
                    for ic, (c0, cw) in enumerate(R_CH):
                        mtiles = []
                        for jb, (j0, jsz) in enumerate(JB):
                            mt = mpool.tile([P, 512], F32, name=f"m{jb}")
                            nc.sync.dma_start(
                                out=mt[:jsz, :cw],
                                in_=maskT[j0:j0 + jsz, c0:c0 + cw])
                            mtiles.append(mt)
                        for ft in range(FT_N):
                            attn_ft(ic, ft, mtiles)
                for ot in range(FT_N):
                    wt = wop.tile([P, KT_N, P], MD, name=f"wo{ot}", tag="wo",
                                  bufs=2)
                    nc.sync.dma_start(out=wt[:], in_=_wslices(woT.ap(), ot * P, P))
                    for rc in range(len(R_CH)):
                        emit_yt(ot, rc, wt)

    nc.compile()
    return nc


W8SCALE = 256.0  # host-side fp8 weight scale (undone at psum eviction)


def build2(score_fp8: bool = False, proj_fp8: bool = True,
           mm_dt=mybir.dt.bfloat16):
    """Causal-only v2: gap-free PE schedule + optional fp8 DoubleRow scores.

    Emission-order discipline: the PE queue is in-order, so every
    potentially-waiting matmul has independent filler work (next-ft Q/K
    projection, V-projection, or output-projection matmuls) queued ahead
    of it.  Fillers are drained from a FIFO of generators, with a burst
    at every (ft, chunk) boundary so the softmax-normalize tail and the
    next chunk's first exp are fully hidden.  PE gaps also reset the PE
    clock ramp (0.65/1.2/2.4 GHz p-states), so continuity matters twice.

    Normalize uses one K=2 rank-2 matmul per (ft, chunk) broadcasting
    both halves' row-sums (f32r), with the reciprocal on a [128, cw]
    tile.  Diagonal-block causal masking runs on the idle Pool engine
    via affine_select; row-sum copies also go to Pool.

    score_fp8: Q/K are evicted to fp8e4 in DoubleRow-packed layout
    ([32 partitions, 2 k-planes]) and scores run as fp8 DoubleRow
    matmuls at 0.5 cycles/column.
    """
    KT_N = E // P
    FT_N = E // P
    R_CH = _chunks(S, 512)
    JB = _chunks(S, P)
    H_LOC = E // D
    nc = bacc.Bacc("TRN2", target_bir_lowering=False, debug=False, num_devices=8)
    MD = mm_dt
    F8 = mybir.dt.float8e4
    NRM = mybir.dt.float32r

    xT = nc.dram_tensor("xT", [E, S], MD, kind="ExternalInput")
    if proj_fp8:
        # DoubleRow fp8 Q/K projections: x8[p, kt, i] = x[kt*128+p, i],
        # w{q,k}8[p, kt, f] = W^T[kt*128+p, f] * W8SCALE
        x8d = nc.dram_tensor("x8", [P, KT_N, S], F8, kind="ExternalInput")
        wq8d = nc.dram_tensor("wq8", [P, KT_N, E], F8, kind="ExternalInput")
        wk8d = nc.dram_tensor("wk8", [P, KT_N, E], F8, kind="ExternalInput")
    else:
        wqT = nc.dram_tensor("wqT", [E, E], MD, kind="ExternalInput")
        wkT = nc.dram_tensor("wkT", [E, E], MD, kind="ExternalInput")
    wvT = nc.dram_tensor("wvT", [E, E], MD, kind="ExternalInput")
    woT = nc.dram_tensor("woT", [E, E], MD, kind="ExternalInput")
    bq = nc.dram_tensor("bq", [E], F32, kind="ExternalInput")
    bo = nc.dram_tensor("bo", [E], F32, kind="ExternalInput")
    yT = nc.dram_tensor("yT", [E, S], F32, kind="ExternalOutput")

    nc._allow_low_precision_reason = "low-precision matmul operand chain"
    with tile.TileContext(nc) as tc:
        with (
            tc.tile_pool(name="persist", bufs=1) as pers,
            tc.tile_pool(name="wpool", bufs=1) as wpool,
            tc.tile_pool(name="attn", bufs=3) as apool,
            tc.tile_pool(name="small", bufs=2) as spool,
            tc.tile_pool(name="evp", bufs=3) as evp,
            tc.tile_pool(name="psP", bufs=1, space="PSUM") as psP,
            tc.tile_pool(name="psS", bufs=1, space="PSUM") as psS,
            tc.tile_pool(name="psO", bufs=1, space="PSUM") as psO,
        ):
            # ---- persistent small tiles / DMA preload (priority order) ----
            bq_sb = pers.tile([P, FT_N], F32, name="bq_sb")
            nc.sync.dma_start(out=bq_sb[:], in_=bq.ap().rearrange("(t p) -> p t", p=P))
            bo_sb = pers.tile([P, FT_N], F32, name="bo_sb")
            nc.sync.dma_start(out=bo_sb[:], in_=bo.ap().rearrange("(t p) -> p t", p=P))

            # Q/K weights streamed per-ft (4 rotating slots, DMA issued
            # ~2 chunks ahead of first use)
            wqk_t: dict = {}
            W8DT = F8 if proj_fp8 else MD

            def dma_wqk(ft):
                if ft in wqk_t or ft >= FT_N:
                    return
                tq = wpool.tile([P, KT_N, P], W8DT, name=f"wq{ft}", tag="wqk",
                                bufs=4)
                tk = wpool.tile([P, KT_N, P], W8DT, name=f"wk{ft}", tag="wqk",
                                bufs=4)
                if proj_fp8:
                    nc.sync.dma_start(out=tq[:],
                                      in_=wq8d[:, :, ft * P:(ft + 1) * P])
                    nc.sync.dma_start(out=tk[:],
                                      in_=wk8d[:, :, ft * P:(ft + 1) * P])
                else:
                    nc.sync.dma_start(out=tq[:], in_=_wslices(wqT.ap(), ft * P, P))
                    nc.sync.dma_start(out=tk[:], in_=_wslices(wkT.ap(), ft * P, P))
                wqk_t[ft] = (tq, tk)

            dma_wqk(0)

            # xT and wv interleaved per-kt (startup critical path)
            XT = [pers.tile([P, S], MD, name=f"xt{kt}") for kt in range(KT_N)]
            fchunks = _chunks(E, 512)
            hpc = 512 // D
            wv_tiles = [wpool.tile([P, KT_N, 512], MD, name=f"wv{fc}")
                        for fc in range(len(fchunks))]
            X8 = None
            if proj_fp8:
                X8 = pers.tile([P, KT_N, S], F8, name="x8sb")
            for kt in range(KT_N):
                nc.sync.dma_start(out=XT[kt][:], in_=xT[kt * P:(kt + 1) * P, :])
                if proj_fp8:
                    nc.sync.dma_start(out=X8[:, kt, :], in_=x8d[:, kt, :])
                for fc, (f0, fw) in enumerate(fchunks):
                    nc.sync.dma_start(
                        out=wv_tiles[fc][:, kt, :fw],
                        in_=_wslices(wvT.ap(), f0, fw)[:, kt, :])
            dma_wqk(1)
            # output-projection weights last
            wo_tiles = [wpool.tile([P, KT_N, P], MD, name=f"wo{ot}")
                        for ot in range(FT_N)]
            for ot in range(FT_N):
                nc.sync.dma_start(out=wo_tiles[ot][:], in_=_wslices(woT.ap(), ot * P, P))

            # sel2: [2, 128], row0 = 1 on cols 0-63, row1 = 1 on cols 64-127
            sel2f = pers.tile([2, P], F32, name="sel2f")
            nc.gpsimd.memset(sel2f[:], 1.0)
            nc.gpsimd.affine_select(
                out=sel2f[:], in_=sel2f[:],
                pattern=[[1, P]], compare_op=mybir.AluOpType.is_ge,
                fill=0.0, base=0, channel_multiplier=-D,
            )
            nc.gpsimd.affine_select(
                out=sel2f[:], in_=sel2f[:],
                pattern=[[-1, P]], compare_op=mybir.AluOpType.is_ge,
                fill=0.0, base=D - 1, channel_multiplier=D,
            )
            sel2 = pers.tile([2, P], NRM, name="sel2")
            nc.vector.tensor_copy(out=sel2[:], in_=sel2f[:])

            # upper-triangular (incl diag) 0/1 mask for diagonal attn blocks
            tri32 = pers.tile([P, P], F32, name="tri32")
            nc.gpsimd.memset(tri32[:], 1.0)
            nc.gpsimd.affine_select(
                out=tri32[:], in_=tri32[:],
                pattern=[[1, P]], compare_op=mybir.AluOpType.is_ge,
                fill=0.0, base=0, channel_multiplier=-1,
            )
            tri = pers.tile([P, P], MD, name="tri")
            nc.vector.tensor_copy(out=tri[:], in_=tri32[:])

            VA = [pers.tile([P, H_LOC * (D + 1)], MD, name=f"va{rt}")
                  for rt in range(len(JB))]
            AOT = [pers.tile([P, S], MD, name=f"aot{ft}") for ft in range(FT_N)]
            if score_fp8:
                # DoubleRow-packed Q/K per ft, fp8.  Partitions [32h:32h+32)
                # hold head h's 64 dims as 2 k-planes (d = 32*plane + p).
                # Q is the moving operand (no stride restriction): [2, S].
                # K is the stationary operand — LDWEIGHTS dual-fp8 requires a
                # power-of-two plane stride, so K is blocked per j-block of
                # 128 with padding: [12, 2, 128].
                Q8 = [pers.tile([P, 2, S], F8, name=f"q8_{ft}")
                      for ft in range(FT_N)]
                K8 = [pers.tile([P, len(JB), 2, P], F8, name=f"k8_{ft}")
                      for ft in range(FT_N)]
            else:
                QT = [pers.tile([P, S], MD, name=f"qt{ft}") for ft in range(FT_N)]
                KTs = [pers.tile([P, S], MD, name=f"kt{ft}") for ft in range(FT_N)]

            # ---------------- work generators (PE-quantum yields) --------
            v_done = [0]  # highest V block index fully emitted + 1

            def gen_v(rts):
                for rt in rts:
                    r0, rsz = JB[rt]
                    for fc, (f0, fw) in enumerate(fchunks):
                        ps = psP.tile([P, 512], F32, name="pv", tag="pp", bufs=2)
                        for kt in range(KT_N):
                            nc.tensor.matmul(
                                ps[:rsz, :fw],
                                XT[kt][:, r0:r0 + rsz],
                                wv_tiles[fc][:, kt, :fw],
                                start=(kt == 0), stop=(kt == KT_N - 1),
                            )
                            yield
                        dst = VA[rt][:].rearrange("p (h c) -> p h c", c=D + 1)
                        nc.vector.tensor_copy(
                            out=dst[:rsz, fc * hpc:fc * hpc + fw // D, 0:D],
                            in_=ps[:rsz, :fw].rearrange("p (h d) -> p h d", d=D),
                        )
                    va3 = VA[rt][:].rearrange("p (h c) -> p h c", c=D + 1)
                    nc.gpsimd.memset(va3[:rsz, :, D:D + 1], 1.0)
                    v_done[0] = rt + 1

            p_done = [0] * FT_N  # chunks (q+k pairs) emitted per ft

            def gen_projqk(ft):
                for rc, (c0, cw) in enumerate(R_CH):
                    for which in range(2):  # 0=q, 1=k
                        wt = wqk_t[ft][which]
                        ps = psP.tile([P, 512], F32, name="pp", tag="pp", bufs=2)
                        if proj_fp8:
                            # DoubleRow: one instruction contracts 2 k-tiles
                            # (256 rows); split columns in half for finer
                            # filler quanta
                            nch = (cw + 255) // 256
                            for ch in range(nch):
                                cl = ch * 256
                                chw = min(256, cw - cl)
                                for t in range(KT_N // 2):
                                    nc.tensor.matmul(
                                        ps[:, cl:cl + chw],
                                        wt[:, 2 * t:2 * t + 2, :],
                                        X8[:, 2 * t:2 * t + 2,
                                           c0 + cl:c0 + cl + chw],
                                        start=(t == 0),
                                        stop=(t == KT_N // 2 - 1),
                                        perf_mode=mybir.MatmulPerfMode.DoubleRow,
                                    )
                                    yield
                        else:
                            for kt in range(KT_N):
                                nc.tensor.matmul(
                                    ps[:, :cw],
                                    wt[:, kt, :],
                                    XT[kt][:, c0:c0 + cw],
                                    start=(kt == 0), stop=(kt == KT_N - 1),
                                )
                                yield
                        inv = 1.0 / W8SCALE if proj_fp8 else None
                        if score_fp8:
                            stg = spool.tile([P, 512], F8, name="stg", tag="stg",
                                             bufs=2)
                            if which == 0:
                                nc.vector.tensor_scalar(
                                    out=stg[:, :cw], in0=ps[:, :cw],
                                    scalar1=inv if inv else 1.0,
                                    scalar2=bq_sb[:, ft:ft + 1],
                                    op0=mybir.AluOpType.mult,
                                    op1=mybir.AluOpType.add,
                                )
                            else:
                                nc.vector.tensor_scalar(
                                    out=stg[:, :cw], in0=ps[:, :cw],
                                    scalar1=inv if inv else 1.0, scalar2=None,
                                    op0=mybir.AluOpType.mult,
                                )
                            for h in range(2):
                                for j in range(2):
                                    src = stg[64 * h + 32 * j:
                                              64 * h + 32 * (j + 1), :cw]
                                    if which == 0:
                                        nc.sync.dma_start(
                                            out=Q8[ft][32 * h:32 * (h + 1),
                                                       j, c0:c0 + cw],
                                            in_=src)
                                    else:
                                        # K: into 128-col padded blocks
                                        b0 = 4 * rc
                                        nfull = cw // P
                                        nc.sync.dma_start(
                                            out=K8[ft][32 * h:32 * (h + 1),
                                                       b0:b0 + nfull, j, :],
                                            in_=src[:, :nfull * P].rearrange(
                                                "p (b c) -> p b c", c=P))
                                        if cw % P:
                                            nc.sync.dma_start(
                                                out=K8[ft][32 * h:32 * (h + 1),
                                                           b0 + nfull, j,
                                                           :cw % P],
                                                in_=src[:, nfull * P:cw])
                        else:
                            dst = QT if which == 0 else KTs
                            if which == 0:
                                if inv is not None:
                                    nc.vector.tensor_scalar(
                                        out=dst[ft][:, c0:c0 + cw],
                                        in0=ps[:, :cw],
                                        scalar1=inv,
                                        scalar2=bq_sb[:, ft:ft + 1],
                                        op0=mybir.AluOpType.mult,
                                        op1=mybir.AluOpType.add,
                                    )
                                else:
                                    nc.vector.tensor_scalar(
                                        out=dst[ft][:, c0:c0 + cw],
                                        in0=ps[:, :cw],
                                        scalar1=bq_sb[:, ft:ft + 1],
                                        scalar2=None,
                                        op0=mybir.AluOpType.add,
                                    )
                            elif inv is not None:
                                nc.vector.tensor_scalar(
                                    out=dst[ft][:, c0:c0 + cw], in0=ps[:, :cw],
                                    scalar1=inv, scalar2=None,
                                    op0=mybir.AluOpType.mult,
                                )
                            else:
                                nc.vector.tensor_copy(
                                    out=dst[ft][:, c0:c0 + cw], in_=ps[:, :cw])
                    p_done[ft] = rc + 1

            def gen_yt(rc):
                c0, cw = R_CH[rc]
                for ot in range(FT_N):
                    psy = psP.tile([P, 512], F32, name="py", tag="pp", bufs=2)
                    for ft in range(FT_N):
                        nc.tensor.matmul(
                            psy[:, :cw],
                            wo_tiles[ot][:, ft, :],
                            AOT[ft][:, c0:c0 + cw],
                            start=(ft == 0), stop=(ft == FT_N - 1),
                        )
                        yield
                    ytt = evp.tile([P, 512], F32, name="yt", tag="yt")
                    nc.vector.tensor_scalar(
                        out=ytt[:, :cw], in0=psy[:, :cw],
                        scalar1=bo_sb[:, ot:ot + 1], scalar2=None,
                        op0=mybir.AluOpType.add,
                    )
                    nc.sync.dma_start(
                        out=yT[ot * P:(ot + 1) * P, c0:c0 + cw], in_=ytt[:, :cw])

            # ---------------- filler FIFO ---------------------------------
            fillers: list = []

            def pump(n=1):
                while n > 0 and fillers:
                    try:
                        next(fillers[0])
                        n -= 1
                    except StopIteration:
                        fillers.pop(0)

            def drain(g):
                if g is None:
                    return
                for _ in g:
                    pass
                if fillers and fillers[0] is g:
                    fillers.pop(0)

            def ensure(g, get, target):
                while get() < target:
                    try:
                        next(g)
                    except StopIteration:
                        break

            # ---------------- attention -----------------------------------
            pending_norm = [None]

            def do_norm():
                fn = pending_norm[0]
                pending_norm[0] = None
                if fn is not None:
                    fn()

            def attn_chunk(ft, ic):
                c0, cw = R_CH[ic]
                nblk = min(len(JB), (c0 + cw + P - 1) // P)
                # natural order = diagonal blocks last, so their extra
                # mask-latency lands at the chunk end where the deferred
                # normalize + next boundary burst hide it
                order = list(range(nblk))
                pso = [psO.tile([D + 1, 512], F32, name=f"po{h}", tag="po", bufs=2)
                       for h in range(2)]
                for n_i, jb in enumerate(order):
                    j0, jsz = JB[jb]
                    vo = max(0, j0 - c0)
                    cwv = cw - vo
                    psp = psS.tile([P, 2, 512], F32, name="psp", tag="ps", bufs=2)
                    if score_fp8:
                        for h in range(2):
                            nc.tensor.matmul(
                                psp[:jsz, h, vo:cw],
                                K8[ft][32 * h:32 * (h + 1), jb, :, :jsz],
                                Q8[ft][32 * h:32 * (h + 1), :,
                                       c0 + vo:c0 + cw],
                                start=True, stop=True,
                                perf_mode=mybir.MatmulPerfMode.DoubleRow,
                                tile_position=(32 * h, 0),
                            )
                    else:
                        for h in range(2):
                            d0 = D * h
                            nc.tensor.matmul(
                                psp[:jsz, h, vo:cw],
                                KTs[ft][d0:d0 + D, j0:j0 + jsz],
                                QT[ft][d0:d0 + D, c0 + vo:c0 + cw],
                                start=True, stop=True,
                                tile_position=(d0, 0),
                            )
                    if n_i == 0:
                        # boundary: the ensure_* burst was just emitted ahead
                        # of us; a little more filler hides the normalize
                        # tail and this chunk's first exp
                        pump(2)
                        do_norm()
                        pump(6)
                    else:
                        pump(1)
                    atp = apool.tile([P, 2, 512], MD, name="atp")
                    nc.scalar.activation(
                        out=atp[:jsz, :, vo:cw], in_=psp[:jsz, :, vo:cw],
                        func=mybir.ActivationFunctionType.Exp,
                    )
                    if j0 >= c0:
                        # zero attn where j > i on the diagonal square
                        for h in range(2):
                            nc.vector.tensor_tensor(
                                out=atp[:jsz, h, vo:vo + jsz],
                                in0=atp[:jsz, h, vo:vo + jsz],
                                in1=tri[:jsz, :jsz],
                                op=mybir.AluOpType.mult,
                            )
                    va3 = VA[jb][:].rearrange("p (h c) -> p h c", c=D + 1)
                    for h in range(2):
                        nc.tensor.matmul(
                            pso[h][:, vo:cw],
                            va3[:jsz, 2 * ft + h, :],
                            atp[:jsz, h, vo:cw],
                            start=(n_i == 0), stop=(n_i == nblk - 1),
                        )
                # row-sums: DVE can only write partition 0, so both halves'
                # rows land flat on partition 0, then one SBUF->SBUF DMA
                # spreads them across partitions 0-1 for the K=2 broadcast
                # matmul; the rest of the normalize is deferred
                ssumf = spool.tile([1, 2, 512], NRM, name="ssumf", tag="ssumf")
                for h in range(2):
                    nc.vector.tensor_copy(
                        out=ssumf[0:1, h, :cw], in_=pso[h][D:D + 1, :cw])
                ssum2 = spool.tile([2, 512], NRM, name="ssum2", tag="ssum2")
                nc.sync.dma_start(out=ssum2[0:2, :cw], in_=ssumf[0:1, :, :cw])

                def norm(ft=ft, ic=ic, pso=pso, ssum2=ssum2, c0=c0, cw=cw):
                    psb = psP.tile([P, 512], F32, name="psb", tag="pp", bufs=2)
                    nc.tensor.matmul(
                        psb[:, :cw], sel2[:, :], ssum2[:, :cw],
                        start=True, stop=True,
                    )
                    rb = spool.tile([P, 512], F32, name="rb", tag="rb", bufs=2)
                    nc.vector.reciprocal_approx_fast(out=rb[:, :cw], in_=psb[:, :cw])
                    for h in range(2):
                        nc.vector.tensor_tensor(
                            out=AOT[ft][D * h:D * (h + 1), c0:c0 + cw],
                            in0=pso[h][0:D, :cw], in1=rb[D * h:D * (h + 1), :cw],
                            op=mybir.AluOpType.mult,
                        )
                pending_norm[0] = norm

            # ---------------- main schedule -------------------------------
            # Startup: V blocks for chunk 0, then Q/K for ft 0 (their
            # matmuls wait on the per-kt xT DMAs, pipelining the load).
            nb0 = min(len(JB), (R_CH[0][0] + R_CH[0][1] + P - 1) // P)
            vgen = gen_v(range(len(JB)))
            ensure(vgen, lambda: v_done[0], nb0)
            projgens = [gen_projqk(ftn) for ftn in range(FT_N)]
            drain(projgens[0])
            fillers.append(vgen)
            for ftn in range(1, FT_N):
                fillers.append(projgens[ftn])

            for ft in range(FT_N):
                for ic in range(len(R_CH)):
                    # demand-driven bursts: everything this chunk's attn
                    # reads must be emitted before it, and the burst doubles
                    # as the boundary filler hiding the normalize tail
                    if ft == 0:
                        c0n, cwn = R_CH[ic]
                        nbn = min(len(JB), (c0n + cwn + P - 1) // P)
                        ensure(vgen, lambda: v_done[0], nbn)
                    if ic == 0 and 1 <= ft:
                        dma_wqk(ft + 1)
                    if ft + 1 < FT_N and ic == len(R_CH) - 1:
                        # pre-emit next ft's first proj chunk before our
                        # last chunk so the ft boundary has cover too
                        ensure(projgens[ft + 1], lambda: p_done[ft + 1], 1)
                    if ft > 0:
                        ensure(projgens[ft], lambda: p_done[ft],
                               min(ic + 2, len(R_CH)))
                    attn_chunk(ft, ic)
                    if ft == FT_N - 1:
                        # yt(ic)'s first 7 accumulations per ot only touch
                        # AOT[0..6] — pre-emit a few as cover, then the
                        # normalize, then the rest follows behind it
                        fillers.append(gen_yt(ic))
                        pump(7)
                        do_norm()
            while fillers:
                pump(10000)

    nc.compile()
    return nc


_CACHE: dict = {}


def _get_nc(causal: bool):
    if causal not in _CACHE:
        _CACHE[causal] = build2() if causal else build(causal)
    return _CACHE[causal]


def _is_causal(mask: np.ndarray) -> bool:
    if mask.shape != (S, S):
        return False
    expect = np.where(np.tril(np.ones((S, S), dtype=bool)), np.float32(0.0),
                      np.float32(NEG))
    return bool(np.array_equal(mask, expect))


MM_NP = ml_dtypes.bfloat16  # numpy dtype matching build()'s default mm_dt


def prep_inputs(x, mask, Wq, bq, Wk, Wv, bv, Wo, bo):
    """Host-side preprocessing shared by kernel() and the bench harness."""
    import ml_dtypes as mld
    scale = np.float32(1.0 / np.sqrt(D))
    xTf = np.ascontiguousarray(np.transpose(x, (0, 2, 1)).astype(np.float32))
    xT = xTf.astype(MM_NP)
    wqTf = np.ascontiguousarray((Wq.astype(np.float32) * scale).T)
    wkTf = np.ascontiguousarray(Wk.astype(np.float32).T)
    common = {
        "wqT": wqTf.astype(MM_NP),
        "wkT": wkTf.astype(MM_NP),
        "wvT": np.ascontiguousarray(Wv.astype(np.float32).T).astype(MM_NP),
        "woT": np.ascontiguousarray(Wo.astype(np.float32).T).astype(MM_NP),
        "bq": (bq.astype(np.float32) * scale),
        "bo": (bo.astype(np.float32) + Wo.astype(np.float32) @ bv.astype(np.float32)),
        # fp8 DoubleRow projection operands (scaled to dodge e4m3 denormals)
        "wq8": np.ascontiguousarray(
            (wqTf * W8SCALE).reshape(8, P, E).transpose(1, 0, 2)
        ).astype(mld.float8_e4m3fn),
        "wk8": np.ascontiguousarray(
            (wkTf * W8SCALE).reshape(8, P, E).transpose(1, 0, 2)
        ).astype(mld.float8_e4m3fn),
    }
    x8 = np.ascontiguousarray(
        xTf.reshape(B, 8, P, S).transpose(0, 2, 1, 3)
    ).astype(mld.float8_e4m3fn)
    causal = _is_causal(np.asarray(mask))
    if not causal:
        common["maskT"] = np.ascontiguousarray(np.asarray(mask, np.float32).T)
    in_maps = [dict(common, xT=xT[b], x8=x8[b]) for b in range(B)]
    return causal, in_maps


_RUNNER: dict = {}


def _get_runner(causal: bool):
    """Compile once per mask-variant; cache the jitted SPMD executable."""
    if causal in _RUNNER:
        return _RUNNER[causal]
    import jax
    from jax.sharding import Mesh, PartitionSpec, NamedSharding
    import warnings
    with warnings.catch_warnings():
        warnings.simplefilter("ignore")
        from jax.experimental.shard_map import shard_map
    from concourse import bass2jax
    from concourse.bass2jax import _bass_exec_p, install_neuronx_cc_hook

    nc = _get_nc(causal)
    install_neuronx_cc_hook()
    partition_name = (nc.partition_id_tensor.name
                      if nc.partition_id_tensor else None)
    in_names, out_names, out_avals = [], [], []
    for alloc in nc.m.functions[0].allocations:
        if not isinstance(alloc, mybir.MemoryLocationSet):
            continue
        name = alloc.memorylocations[0].name
        if alloc.kind == "ExternalInput":
            if name != partition_name:
                in_names.append(name)
        elif alloc.kind == "ExternalOutput":
            out_names.append(name)
            out_avals.append(jax.core.ShapedArray(
                tuple(alloc.tensor_shape), mybir.dt.np(alloc.dtype)))
    n_params = len(in_names)
    n_outs = len(out_names)

    def _body(*args):
        operands = list(args)
        names = list(in_names) + list(out_names)
        if partition_name is not None:
            operands.append(bass2jax.partition_id_tensor())
            names.append(partition_name)
        outs = _bass_exec_p.bind(
            *operands,
            out_avals=tuple(out_avals),
            in_names=tuple(names),
            out_names=tuple(out_names),
            lowering_input_output_aliases=(),
            sim_require_finite=True,
            sim_require_nnan=True,
            nc=nc,
        )
        return tuple(outs)

    devices = jax.devices()[:B]
    mesh = Mesh(np.asarray(devices), ("core",))
    in_specs = (PartitionSpec("core"),) * (n_params + n_outs)
    out_specs = (PartitionSpec("core"),) * n_outs
    fn = jax.jit(
        shard_map(_body, mesh=mesh, in_specs=in_specs, out_specs=out_specs,
                  check_rep=False),
        donate_argnums=tuple(range(n_params, n_params + n_outs)),
        keep_unused=True,
    )
    runner = (fn, in_names, out_names, out_avals)
    _RUNNER[causal] = runner
    return runner


def kernel(x, mask, Wq, bq, Wk, Wv, bv, Wo, bo):
    causal, in_maps = prep_inputs(x, mask, Wq, bq, Wk, Wv, bv, Wo, bo)
    fn, in_names, out_names, out_avals = _get_runner(causal)
    cat = [np.concatenate([np.asarray(m[n]) for m in in_maps], axis=0)
           for n in in_names]
    zs = [np.zeros((B * a.shape[0], *a.shape[1:]), a.dtype) for a in out_avals]
    outs = fn(*cat, *zs)
    yT = np.asarray(outs[out_names.index("yT")]).reshape(B, E, S)
    out = np.ascontiguousarray(yT.transpose(0, 2, 1).astype(np.float32))
    return out



# revision 46
# speedup vs baseline: 1.1805x; 1.1456x over previous
"""Multi-head attention (B=8, S=1500, E=1024, H=16, D=64) on 8 trn2 NeuronCores.

Sharding: pure data-parallel over batch — core b computes batch element b
end-to-end (no collectives). Host pre-transposes x and the weights so every
device-side matmul has its contraction dim on the SBUF partition axis, and
folds the 1/sqrt(D) scale into Wq/bq and the V-bias into the output bias
(bo_eff = bo + Wo @ bv), so the device kernel never touches bv.

Device pipeline per core (all f32 storage, matmuls run as float32r):
  QT = (Wq/8)^T-proj of x^T   [1024, 1500]  (f-on-partition; bias bq/8 per-partition)
  KT = Wk^T-proj              [1024, 1500]
  V_aug = x @ Wv^T with a ones-column appended per head  [1500, 16*65]
  per (i-chunk, head): scoresT[j, i] via matmul(lhsT=KT_h, rhs=QT_h);
    exp on ACT (no max-subtraction: |scores| <~ 30, safely inside f32);
    causal masking structurally (affine_select zero-fill on diagonal blocks)
    or via an additive mask tensor (general path);
  out^T + rowsums in ONE matmul: lhsT=[V_h | 1] (65 cols), rhs=attnT;
  normalize: recip of the sums row, rank-1 matmul broadcast across 64
    partitions, multiply on eviction -> AO^T;
  yT = Wo^T-proj of AO^T + bo_eff  -> DRAM [1024, 1500], host transposes back.
"""

import sys
import numpy as np
import ml_dtypes

for _p in ("/opt/trn_rl_repo",):
    if _p not in sys.path:
        sys.path.append(_p)

import concourse.bass as bass
import concourse.mybir as mybir
import concourse.tile as tile
from concourse import bacc
from concourse.bass_utils import run_bass_kernel_spmd

F32 = mybir.dt.float32

B, S, E, H, D = 8, 1500, 1024, 16, 64
P = 128
NEG = -1e9


def _chunks(total, step):
    return [(c0, min(step, total - c0)) for c0 in range(0, total, step)]


def _wslices(dram_ap, col0, cols):
    """[E, E] weight -> [P, E//P, cols] AP for a column slice (k on partition)."""
    return dram_ap.rearrange("(kt p) f -> p kt f", p=P)[:, :, col0:col0 + cols]


def build(causal: bool, mm_dt=mybir.dt.bfloat16):
    KT_N = E // P            # k-tiles over the embedding dim
    FT_N = E // P            # f-tiles
    R_CH = _chunks(S, 512)   # i/r chunks
    JB = _chunks(S, P)       # j blocks
    H_LOC = E // D
    nc = bacc.Bacc("TRN2", target_bir_lowering=False, debug=False, num_devices=8)
    MD = mm_dt  # dtype for every matmul operand chain
    NRM = mybir.dt.float32r if MD == mybir.dt.bfloat16 else MD

    xT = nc.dram_tensor("xT", [E, S], MD, kind="ExternalInput")
    wqT = nc.dram_tensor("wqT", [E, E], MD, kind="ExternalInput")
    wkT = nc.dram_tensor("wkT", [E, E], MD, kind="ExternalInput")
    wvT = nc.dram_tensor("wvT", [E, E], MD, kind="ExternalInput")
    woT = nc.dram_tensor("woT", [E, E], MD, kind="ExternalInput")
    bq = nc.dram_tensor("bq", [E], F32, kind="ExternalInput")
    bo = nc.dram_tensor("bo", [E], F32, kind="ExternalInput")
    maskT = None
    if not causal:
        maskT = nc.dram_tensor("maskT", [S, S], F32, kind="ExternalInput")
    yT = nc.dram_tensor("yT", [E, S], F32, kind="ExternalOutput")

    def mm(ap):
        return ap

    nc._allow_low_precision_reason = "low-precision matmul operand chain"
    with tile.TileContext(nc) as tc:
        with (
            tc.tile_pool(name="persist", bufs=1) as pers,
            tc.tile_pool(name="wqkp", bufs=2) as wqkp,
            tc.tile_pool(name="wvp", bufs=1) as wvp,
            tc.tile_pool(name="wop", bufs=1) as wop,
            tc.tile_pool(name="attn", bufs=3) as apool,
            tc.tile_pool(name="small", bufs=3) as spool,
            tc.tile_pool(name="evp", bufs=3) as evp,
            tc.tile_pool(name="psP", bufs=1, space="PSUM") as psP,
            tc.tile_pool(name="psS", bufs=1, space="PSUM") as psS,
            tc.tile_pool(name="psO", bufs=1, space="PSUM") as psO,
            tc.tile_pool(name="psY", bufs=1, space="PSUM") as psY,
        ):
            ones64 = pers.tile([1, D], NRM, name="ones64")
            nc.vector.memset(ones64[:].bitcast(F32), 1.0)
            bq_sb = pers.tile([P, FT_N], F32, name="bq_sb")
            nc.sync.dma_start(out=bq_sb[:], in_=bq.ap().rearrange("(t p) -> p t", p=P))
            bo_sb = pers.tile([P, FT_N], F32, name="bo_sb")
            nc.sync.dma_start(out=bo_sb[:], in_=bo.ap().rearrange("(t p) -> p t", p=P))

            # upper-triangular (incl diag) 0/1 mask for diagonal attn blocks
            tri32 = pers.tile([P, P], F32, name="tri32")
            nc.gpsimd.memset(tri32[:], 1.0)
            nc.gpsimd.affine_select(
                out=tri32[:], in_=tri32[:],
                pattern=[[1, P]], compare_op=mybir.AluOpType.is_ge,
                fill=0.0, base=0, channel_multiplier=-1,
            )
            tri = pers.tile([P, P], MD, name="tri")
            nc.vector.tensor_copy(out=tri[:], in_=tri32[:])

            XT = [pers.tile([P, S], MD, name=f"xt{kt}") for kt in range(KT_N)]
            QT = [pers.tile([P, S], MD, name=f"qt{ft}") for ft in range(FT_N)]
            KTs = [pers.tile([P, S], MD, name=f"kt{ft}") for ft in range(FT_N)]
            VA = [pers.tile([P, H_LOC * (D + 1)], MD, name=f"va{rt}")
                  for rt in range(len(JB))]
            AOT = [pers.tile([P, S], MD, name=f"aot{ft}") for ft in range(FT_N)]

            # ---- V projection (natural layout, ones column appended) ----
            hpc = 512 // D  # heads per 512-wide f chunk
            fchunks = _chunks(E, 512)
            wv_tiles = [wvp.tile([P, KT_N, 512], MD, name=f"wv{fc}", tag=f"wv{fc}")
                        for fc in range(len(fchunks))]
            # interleave wv-slice and xT-tile loads so the first V matmuls
            # can issue as soon as (wv[:,0,:], xT[0]) land
            for kt in range(KT_N):
                for fc, (f0, fw) in enumerate(fchunks):
                    nc.sync.dma_start(
                        out=wv_tiles[fc][:, kt, :fw],
                        in_=_wslices(wvT.ap(), f0, fw)[:, kt, :])
                nc.sync.dma_start(out=XT[kt][:],
                                  in_=xT[kt * P:(kt + 1) * P, :])

            def emit_v(rts):
                for rt in rts:
                    if rt >= len(JB):
                        continue
                    r0, rsz = JB[rt]
                    for fc, (f0, fw) in enumerate(fchunks):
                        wt = wv_tiles[fc]
                        ps = psP.tile([P, 512], F32, name="pv", tag="pp", bufs=2)
                        for kt in range(KT_N):
                            nc.tensor.matmul(
                                ps[:rsz, :fw],
                                mm(XT[kt][:, r0:r0 + rsz]),
                                mm(wt[:, kt, :fw]),
                                start=(kt == 0), stop=(kt == KT_N - 1),
                            )
                        dst = VA[rt][:].rearrange("p (h c) -> p h c", c=D + 1)
                        nc.vector.tensor_copy(
                            out=dst[:rsz, fc * hpc:fc * hpc + fw // D, 0:D],
                            in_=ps[:rsz, :fw].rearrange("p (h d) -> p h d", d=D),
                        )
                    va3 = VA[rt][:].rearrange("p (h c) -> p h c", c=D + 1)
                    if MD == mybir.dt.float32r:
                        nc.gpsimd.memset(va3[:rsz, :, D:D + 1].bitcast(F32), 1.0)
                    else:
                        nc.gpsimd.memset(va3[:rsz, :, D:D + 1], 1.0)

            def proj_qk_gen(ft):
                for which, wdram, dst in (("q", wqT, QT), ("k", wkT, KTs)):
                    wt = wqkp.tile([P, KT_N, P], MD, name="wqk", tag="wqk")
                    nc.sync.dma_start(out=wt[:], in_=_wslices(wdram.ap(), ft * P, P))
                    for rc, (c0, cw) in enumerate(R_CH):
                        ps = psP.tile([P, 512], F32, name="pp", tag="pp", bufs=2)
                        for kt in range(KT_N):
                            nc.tensor.matmul(
                                ps[:, :cw],
                                mm(wt[:, kt, :]),
                                mm(XT[kt][:, c0:c0 + cw]),
                                start=(kt == 0), stop=(kt == KT_N - 1),
                            )
                        if which == "q":
                            nc.vector.tensor_scalar(
                                out=dst[ft][:, c0:c0 + cw], in0=ps[:, :cw],
                                scalar1=bq_sb[:, ft:ft + 1], scalar2=None,
                                op0=mybir.AluOpType.add,
                            )
                        else:
                            nc.vector.tensor_copy(
                                out=dst[ft][:, c0:c0 + cw], in_=ps[:, :cw])
                        yield

            def proj_qk(ft):
                for _ in proj_qk_gen(ft):
                    pass

            def attn_ft(ic, ft, mtiles, filler=None):
                c0, cw = R_CH[ic]
                nblk = (min(len(JB), (c0 + cw + P - 1) // P)
                        if causal else len(JB))
                pso = [psO.tile([D + 1, 512], F32, name=f"po{half}",
                                tag="po", bufs=2)
                       for half in range(2)]
                # diagonal-containing blocks first so the chunk-end attnV
                # gates on a short (non-masked) exp chain
                if causal:
                    cut = max(0, nblk - (cw + P - 1) // P)
                    order = list(range(cut, nblk)) + list(range(cut))
                else:
                    order = list(range(nblk))
                for n_i, jb in enumerate(order):
                    j0, jsz = JB[jb]
                    vo = max(0, j0 - c0) if causal else 0
                    # both halves' scores land in one 2-bank psum pair so a
                    # single ACTIVATE exps them together (halves ACT op count)
                    psp = psS.tile([P, 2, 512], F32, name="psp",
                                   tag="ps", bufs=2)
                    for half in range(2):
                        d0 = D * half
                        nc.tensor.matmul(
                            psp[:jsz, half, vo:cw],
                            mm(KTs[ft][d0:d0 + D, j0:j0 + jsz]),
                            mm(QT[ft][d0:d0 + D, c0 + vo:c0 + cw]),
                            start=True, stop=True,
                            tile_position=(d0, 0),
                        )
                    if not causal:
                        for half in range(2):
                            nc.vector.tensor_tensor(
                                out=psp[:jsz, half, :cw],
                                in0=psp[:jsz, half, :cw],
                                in1=mtiles[jb][:jsz, :cw],
                                op=mybir.AluOpType.add,
                            )
                    atp = apool.tile([P, 2, 512], MD, name="atp")
                    nc.scalar.activation(
                        out=atp[:jsz, :, vo:cw], in_=psp[:jsz, :, vo:cw],
                        func=mybir.ActivationFunctionType.Exp,
                    )
                    if causal and j0 >= c0:
                        # zero attn where j > i on the diagonal square
                        for half in range(2):
                            nc.vector.tensor_tensor(
                                out=atp[:jsz, half, vo:vo + jsz],
                                in0=atp[:jsz, half, vo:vo + jsz],
                                in1=tri[:jsz, :jsz],
                                op=mybir.AluOpType.mult,
                            )
                    va3 = VA[jb][:].rearrange("p (h c) -> p h c", c=D + 1)
                    for half in range(2):
                        nc.tensor.matmul(
                            pso[half][:, vo:cw],
                            mm(va3[:jsz, 2 * ft + half, :]),
                            mm(atp[:jsz, half, vo:cw]),
                            start=(n_i == 0), stop=(n_i == nblk - 1),
                        )
                    if filler is not None and n_i % 4 == 3:
                        filler()
                ssums = []
                for half in range(2):
                    ssum = spool.tile([1, 512], NRM, name=f"ssum{half}",
                                      tag="ssum")
                    nc.vector.tensor_copy(
                        out=ssum[:, :cw], in_=pso[half][D:D + 1, :cw])
                    ssums.append(ssum)
                for half in range(2):
                    d0 = D * half
                    psb = psP.tile([D, 512], F32, name="psb", tag="pp", bufs=2)
                    nc.tensor.matmul(
                        psb[:, :cw], mm(ones64[:, :]), mm(ssums[half][:, :cw]),
                        start=True, stop=True,
                    )
                    rb = spool.tile([D, 512], F32, name="rb")
                    nc.vector.reciprocal_approx_fast(
                        out=rb[:, :cw], in_=psb[:, :cw])
                    nc.vector.tensor_tensor(
                        out=AOT[ft][d0:d0 + D, c0:c0 + cw],
                        in0=pso[half][0:D, :cw], in1=rb[:, :cw],
                        op=mybir.AluOpType.mult,
                    )

            def emit_yt(ot, rc, wo_t):
                c0, cw = R_CH[rc]
                psy = psP.tile([P, 512], F32, name="py", tag="pp", bufs=2)
                for ft in range(FT_N):
                    nc.tensor.matmul(
                        psy[:, :cw],
                        mm(wo_t[:, ft, :]),
                        mm(AOT[ft][:, c0:c0 + cw]),
                        start=(ft == 0), stop=(ft == FT_N - 1),
                    )
                yt = evp.tile([P, 512], F32, name="yt", tag="yt")
                nc.vector.tensor_scalar(
                    out=yt[:, :cw], in0=psy[:, :cw],
                    scalar1=bo_sb[:, ot:ot + 1], scalar2=None,
                    op0=mybir.AluOpType.add,
                )
                nc.sync.dma_start(
                    out=yT[ot * P:(ot + 1) * P, c0:c0 + cw], in_=yt[:, :cw])

            if causal:
                wo_tiles = []
                for ot in range(FT_N):
                    wt = wop.tile([P, KT_N, P], MD, name=f"wo{ot}",
                                  tag=f"wo{ot}")
                    nc.sync.dma_start(out=wt[:],
                                      in_=_wslices(woT.ap(), ot * P, P))
                    wo_tiles.append(wt)
                nb0 = min(len(JB), (R_CH[0][0] + R_CH[0][1] + P - 1) // P)
                emit_v(range(nb0))
                proj_qk(0)
                nbp = nb0
                for ft in range(FT_N):
                    gen = proj_qk_gen(ft + 1) if ft + 1 < FT_N else None

                    def pump():
                        if gen is not None:
                            next(gen, None)

                    for ic in range(len(R_CH)):
                        attn_ft(ic, ft, None, filler=pump)
                        if ft == 0 and ic + 1 < len(R_CH):
                            c0n, cwn = R_CH[ic + 1]
                            nbn = min(len(JB), (c0n + cwn + P - 1) // P)
                            emit_v(range(nbp, nbn))
                            nbp = nbn
                        if ft == FT_N - 1:
                            # last ft has no next-ft projection filler: use the
                            # now-ready yT chunk as PE filler instead
                            for ot in range(FT_N):
                                emit_yt(ot, ic, wo_tiles[ot])
                    if gen is not None:
                        for _ in gen:
                            pass
            else:
                emit_v(range(len(JB)))
                for ft in range(FT_N):
                    proj_qk(ft)
                with tc.tile_pool(name="maskp", bufs=1) as mpool:
                    for ic, (c0, cw) in enumerate(R_CH):
                        mtiles = []
                        for jb, (j0, jsz) in enumerate(JB):
                            mt = mpool.tile([P, 512], F32, name=f"m{jb}")
                            nc.sync.dma_start(
                                out=mt[:jsz, :cw],
                                in_=maskT[j0:j0 + jsz, c0:c0 + cw])
                            mtiles.append(mt)
                        for ft in range(FT_N):
                            attn_ft(ic, ft, mtiles)
                for ot in range(FT_N):
                    wt = wop.tile([P, KT_N, P], MD, name=f"wo{ot}", tag="wo",
                                  bufs=2)
                    nc.sync.dma_start(out=wt[:], in_=_wslices(woT.ap(), ot * P, P))
                    for rc in range(len(R_CH)):
                        emit_yt(ot, rc, wt)

    nc.compile()
    return nc


W8SCALE = 256.0  # host-side fp8 weight scale (undone at psum eviction)


def build2(score_fp8: bool = False, proj_fp8: bool = True,
           mm_dt=mybir.dt.bfloat16):
    """Causal-only v2: gap-free PE schedule + optional fp8 DoubleRow scores.

    Emission-order discipline: the PE queue is in-order, so every
    potentially-waiting matmul has independent filler work (next-ft Q/K
    projection, V-projection, or output-projection matmuls) queued ahead
    of it.  Fillers are drained from a FIFO of generators, with a burst
    at every (ft, chunk) boundary so the softmax-normalize tail and the
    next chunk's first exp are fully hidden.  PE gaps also reset the PE
    clock ramp (0.65/1.2/2.4 GHz p-states), so continuity matters twice.

    Normalize uses one K=2 rank-2 matmul per (ft, chunk) broadcasting
    both halves' row-sums (f32r), with the reciprocal on a [128, cw]
    tile.  Diagonal-block causal masking runs on the idle Pool engine
    via affine_select; row-sum copies also go to Pool.

    score_fp8: Q/K are evicted to fp8e4 in DoubleRow-packed layout
    ([32 partitions, 2 k-planes]) and scores run as fp8 DoubleRow
    matmuls at 0.5 cycles/column.
    """
    KT_N = E // P
    FT_N = E // P
    R_CH = _chunks(S, 512)
    JB = _chunks(S, P)
    H_LOC = E // D
    nc = bacc.Bacc("TRN2", target_bir_lowering=False, debug=False, num_devices=8)
    MD = mm_dt
    F8 = mybir.dt.float8e4
    NRM = mybir.dt.float32r

    xT = nc.dram_tensor("xT", [E, S], MD, kind="ExternalInput")
    if proj_fp8:
        # DoubleRow fp8 Q/K projections: x8[p, kt, i] = x[kt*128+p, i],
        # w{q,k}8[p, kt, f] = W^T[kt*128+p, f] * W8SCALE
        x8d = nc.dram_tensor("x8", [P, KT_N, S], F8, kind="ExternalInput")
        wq8d = nc.dram_tensor("wq8", [P, KT_N, E], F8, kind="ExternalInput")
        wk8d = nc.dram_tensor("wk8", [P, KT_N, E], F8, kind="ExternalInput")
    else:
        wqT = nc.dram_tensor("wqT", [E, E], MD, kind="ExternalInput")
        wkT = nc.dram_tensor("wkT", [E, E], MD, kind="ExternalInput")
    wvT = nc.dram_tensor("wvT", [E, E], MD, kind="ExternalInput")
    woT = nc.dram_tensor("woT", [E, E], MD, kind="ExternalInput")
    bq = nc.dram_tensor("bq", [E], F32, kind="ExternalInput")
    bo = nc.dram_tensor("bo", [E], F32, kind="ExternalInput")
    yT = nc.dram_tensor("yT", [E, S], F32, kind="ExternalOutput")

    nc._allow_low_precision_reason = "low-precision matmul operand chain"
    with tile.TileContext(nc) as tc:
        with (
            tc.tile_pool(name="persist", bufs=1) as pers,
            tc.tile_pool(name="wpool", bufs=1) as wpool,
            tc.tile_pool(name="attn", bufs=3) as apool,
            tc.tile_pool(name="small", bufs=2) as spool,
            tc.tile_pool(name="evp", bufs=3) as evp,
            tc.tile_pool(name="psP", bufs=1, space="PSUM") as psP,
            tc.tile_pool(name="psS", bufs=1, space="PSUM") as psS,
            tc.tile_pool(name="psO", bufs=1, space="PSUM") as psO,
        ):
            # ---- persistent small tiles / DMA preload (priority order) ----
            bq_sb = pers.tile([P, FT_N], F32, name="bq_sb")
            nc.sync.dma_start(out=bq_sb[:], in_=bq.ap().rearrange("(t p) -> p t", p=P))
            bo_sb = pers.tile([P, FT_N], F32, name="bo_sb")
            nc.sync.dma_start(out=bo_sb[:], in_=bo.ap().rearrange("(t p) -> p t", p=P))

            # Q/K weights streamed per-ft (4 rotating slots, DMA issued
            # ~2 chunks ahead of first use)
            wqk_t: dict = {}
            W8DT = F8 if proj_fp8 else MD

            def dma_wqk(ft):
                if ft in wqk_t or ft >= FT_N:
                    return
                tq = wpool.tile([P, KT_N, P], W8DT, name=f"wq{ft}", tag="wqk",
                                bufs=4)
                tk = wpool.tile([P, KT_N, P], W8DT, name=f"wk{ft}", tag="wqk",
                                bufs=4)
                if proj_fp8:
                    nc.sync.dma_start(out=tq[:],
                                      in_=wq8d[:, :, ft * P:(ft + 1) * P])
                    nc.sync.dma_start(out=tk[:],
                                      in_=wk8d[:, :, ft * P:(ft + 1) * P])
                else:
                    nc.sync.dma_start(out=tq[:], in_=_wslices(wqT.ap(), ft * P, P))
                    nc.sync.dma_start(out=tk[:], in_=_wslices(wkT.ap(), ft * P, P))
                wqk_t[ft] = (tq, tk)

            dma_wqk(0)

            # xT and wv interleaved per-kt (startup critical path)
            XT = [pers.tile([P, S], MD, name=f"xt{kt}") for kt in range(KT_N)]
            fchunks = _chunks(E, 512)
            hpc = 512 // D
            wv_tiles = [wpool.tile([P, KT_N, 512], MD, name=f"wv{fc}")
                        for fc in range(len(fchunks))]
            X8 = None
            if proj_fp8:
                X8 = pers.tile([P, KT_N, S], F8, name="x8sb")
            for kt in range(KT_N):
                nc.sync.dma_start(out=XT[kt][:], in_=xT[kt * P:(kt + 1) * P, :])
                if proj_fp8:
                    nc.sync.dma_start(out=X8[:, kt, :], in_=x8d[:, kt, :])
                for fc, (f0, fw) in enumerate(fchunks):
                    nc.sync.dma_start(
                        out=wv_tiles[fc][:, kt, :fw],
                        in_=_wslices(wvT.ap(), f0, fw)[:, kt, :])
            dma_wqk(1)
            # output-projection weights last
            wo_tiles = [wpool.tile([P, KT_N, P], MD, name=f"wo{ot}")
                        for ot in range(FT_N)]
            for ot in range(FT_N):
                nc.sync.dma_start(out=wo_tiles[ot][:], in_=_wslices(woT.ap(), ot * P, P))

            ones64 = pers.tile([1, D], NRM, name="ones64")
            nc.vector.memset(ones64[:].bitcast(F32), 1.0)

            # upper-triangular (incl diag) 0/1 mask for diagonal attn blocks
            tri32 = pers.tile([P, P], F32, name="tri32")
            nc.gpsimd.memset(tri32[:], 1.0)
            nc.gpsimd.affine_select(
                out=tri32[:], in_=tri32[:],
                pattern=[[1, P]], compare_op=mybir.AluOpType.is_ge,
                fill=0.0, base=0, channel_multiplier=-1,
            )
            tri = pers.tile([P, P], MD, name="tri")
            nc.vector.tensor_copy(out=tri[:], in_=tri32[:])

            VA = [pers.tile([P, H_LOC * (D + 1)], MD, name=f"va{rt}")
                  for rt in range(len(JB))]
            AOT = [pers.tile([P, S], MD, name=f"aot{ft}") for ft in range(FT_N)]
            if score_fp8:
                # DoubleRow-packed Q/K per ft, fp8.  Partitions [32h:32h+32)
                # hold head h's 64 dims as 2 k-planes (d = 32*plane + p).
                # Q is the moving operand (no stride restriction): [2, S].
                # K is the stationary operand — LDWEIGHTS dual-fp8 requires a
                # power-of-two plane stride, so K is blocked per j-block of
                # 128 with padding: [12, 2, 128].
                Q8 = [pers.tile([P, 2, S], F8, name=f"q8_{ft}")
                      for ft in range(FT_N)]
                K8 = [pers.tile([P, len(JB), 2, P], F8, name=f"k8_{ft}")
                      for ft in range(FT_N)]
            else:
                QT = [pers.tile([P, S], MD, name=f"qt{ft}") for ft in range(FT_N)]
                KTs = [pers.tile([P, S], MD, name=f"kt{ft}") for ft in range(FT_N)]

            # ---------------- work generators (PE-quantum yields) --------
            v_done = [0]  # highest V block index fully emitted + 1

            def gen_v(rts):
                for rt in rts:
                    r0, rsz = JB[rt]
                    for fc, (f0, fw) in enumerate(fchunks):
                        ps = psP.tile([P, 512], F32, name="pv", tag="pp", bufs=2)
                        for kt in range(KT_N):
                            nc.tensor.matmul(
                                ps[:rsz, :fw],
                                XT[kt][:, r0:r0 + rsz],
                                wv_tiles[fc][:, kt, :fw],
                                start=(kt == 0), stop=(kt == KT_N - 1),
                            )
                            yield
                        dst = VA[rt][:].rearrange("p (h c) -> p h c", c=D + 1)
                        nc.vector.tensor_copy(
                            out=dst[:rsz, fc * hpc:fc * hpc + fw // D, 0:D],
                            in_=ps[:rsz, :fw].rearrange("p (h d) -> p h d", d=D),
                        )
                    va3 = VA[rt][:].rearrange("p (h c) -> p h c", c=D + 1)
                    nc.gpsimd.memset(va3[:rsz, :, D:D + 1], 1.0)
                    v_done[0] = rt + 1

            p_done = [0] * FT_N  # chunks (q+k pairs) emitted per ft

            def gen_projqk(ft):
                dma_wqk(ft)  # idempotent; normally prefetched much earlier
                for rc, (c0, cw) in enumerate(R_CH):
                    for which in range(2):  # 0=q, 1=k
                        wt = wqk_t[ft][which]
                        ps = psP.tile([P, 512], F32, name="pp", tag="pp", bufs=2)
                        if proj_fp8:
                            # DoubleRow: one instruction contracts 2 k-tiles
                            # (256 rows); split columns in half for finer
                            # filler quanta
                            nch = (cw + 255) // 256
                            for ch in range(nch):
                                cl = ch * 256
                                chw = min(256, cw - cl)
                                for t in range(KT_N // 2):
                                    nc.tensor.matmul(
                                        ps[:, cl:cl + chw],
                                        wt[:, 2 * t:2 * t + 2, :],
                                        X8[:, 2 * t:2 * t + 2,
                                           c0 + cl:c0 + cl + chw],
                                        start=(t == 0),
                                        stop=(t == KT_N // 2 - 1),
                                        perf_mode=mybir.MatmulPerfMode.DoubleRow,
                                    )
                                    yield
                        else:
                            for kt in range(KT_N):
                                nc.tensor.matmul(
                                    ps[:, :cw],
                                    wt[:, kt, :],
                                    XT[kt][:, c0:c0 + cw],
                                    start=(kt == 0), stop=(kt == KT_N - 1),
                                )
                                yield
                        inv = 1.0 / W8SCALE if proj_fp8 else None
                        if score_fp8:
                            stg = spool.tile([P, 512], F8, name="stg", tag="stg",
                                             bufs=2)
                            if which == 0:
                                nc.vector.tensor_scalar(
                                    out=stg[:, :cw], in0=ps[:, :cw],
                                    scalar1=inv if inv else 1.0,
                                    scalar2=bq_sb[:, ft:ft + 1],
                                    op0=mybir.AluOpType.mult,
                                    op1=mybir.AluOpType.add,
                                )
                            else:
                                nc.vector.tensor_scalar(
                                    out=stg[:, :cw], in0=ps[:, :cw],
                                    scalar1=inv if inv else 1.0, scalar2=None,
                                    op0=mybir.AluOpType.mult,
                                )
                            for h in range(2):
                                for j in range(2):
                                    src = stg[64 * h + 32 * j:
                                              64 * h + 32 * (j + 1), :cw]
                                    if which == 0:
                                        nc.sync.dma_start(
                                            out=Q8[ft][32 * h:32 * (h + 1),
                                                       j, c0:c0 + cw],
                                            in_=src)
                                    else:
                                        # K: into 128-col padded blocks
                                        b0 = 4 * rc
                                        nfull = cw // P
                                        nc.sync.dma_start(
                                            out=K8[ft][32 * h:32 * (h + 1),
                                                       b0:b0 + nfull, j, :],
                                            in_=src[:, :nfull * P].rearrange(
                                                "p (b c) -> p b c", c=P))
                                        if cw % P:
                                            nc.sync.dma_start(
                                                out=K8[ft][32 * h:32 * (h + 1),
                                                           b0 + nfull, j,
                                                           :cw % P],
                                                in_=src[:, nfull * P:cw])
                        else:
                            dst = QT if which == 0 else KTs
                            if which == 0:
                                if inv is not None:
                                    nc.vector.tensor_scalar(
                                        out=dst[ft][:, c0:c0 + cw],
                                        in0=ps[:, :cw],
                                        scalar1=inv,
                                        scalar2=bq_sb[:, ft:ft + 1],
                                        op0=mybir.AluOpType.mult,
                                        op1=mybir.AluOpType.add,
                                    )
                                else:
                                    nc.vector.tensor_scalar(
                                        out=dst[ft][:, c0:c0 + cw],
                                        in0=ps[:, :cw],
                                        scalar1=bq_sb[:, ft:ft + 1],
                                        scalar2=None,
                                        op0=mybir.AluOpType.add,
                                    )
                            elif inv is not None:
                                nc.vector.tensor_scalar(
                                    out=dst[ft][:, c0:c0 + cw], in0=ps[:, :cw],
                                    scalar1=inv, scalar2=None,
                                    op0=mybir.AluOpType.mult,
                                )
                            else:
                                nc.vector.tensor_copy(
                                    out=dst[ft][:, c0:c0 + cw], in_=ps[:, :cw])
                    p_done[ft] = rc + 1

            def gen_yt(rc):
                c0, cw = R_CH[rc]
                for ot in range(FT_N):
                    psy = psP.tile([P, 512], F32, name="py", tag="pp", bufs=2)
                    for ft in range(FT_N):
                        nc.tensor.matmul(
                            psy[:, :cw],
                            wo_tiles[ot][:, ft, :],
                            AOT[ft][:, c0:c0 + cw],
                            start=(ft == 0), stop=(ft == FT_N - 1),
                        )
                        yield
                    ytt = evp.tile([P, 512], F32, name="yt", tag="yt")
                    nc.vector.tensor_scalar(
                        out=ytt[:, :cw], in0=psy[:, :cw],
                        scalar1=bo_sb[:, ot:ot + 1], scalar2=None,
                        op0=mybir.AluOpType.add,
                    )
                    nc.sync.dma_start(
                        out=yT[ot * P:(ot + 1) * P, c0:c0 + cw], in_=ytt[:, :cw])

            # ---------------- filler FIFO ---------------------------------
            fillers: list = []

            def pump(n=1, dummy=0):
                # pull up to n real filler quanta; if the FIFO runs dry,
                # emit up to `dummy` no-op LDWEIGHTS — they keep the PE
                # executing (clock p-state) across dependency waits and are
                # harmless since every matmul reloads its own weights
                while n > 0 and fillers:
                    try:
                        next(fillers[0])
                        n -= 1
                    except StopIteration:
                        fillers.pop(0)
                for _ in range(min(n, dummy)):
                    nc.tensor.ldweights(tri[:, :])

            def drain(g):
                if g is None:
                    return
                for _ in g:
                    pass
                if fillers and fillers[0] is g:
                    fillers.pop(0)

            def ensure(g, get, target):
                while get() < target:
                    try:
                        next(g)
                    except StopIteration:
                        break

            # ---------------- attention -----------------------------------
            pending_norm = [None]

            def do_norm():
                fn = pending_norm[0]
                pending_norm[0] = None
                if fn is not None:
                    fn()

            def attn_chunk(ft, ic):
                c0, cw = R_CH[ic]
                nblk = min(len(JB), (c0 + cw + P - 1) // P)
                # natural order = diagonal blocks last, so their extra
                # mask-latency lands at the chunk end where the deferred
                # normalize + next boundary burst hide it
                order = list(range(nblk))
                pso = [psO.tile([D + 1, 512], F32, name=f"po{h}", tag="po", bufs=2)
                       for h in range(2)]
                for n_i, jb in enumerate(order):
                    j0, jsz = JB[jb]
                    vo = max(0, j0 - c0)
                    cwv = cw - vo
                    psp = psS.tile([P, 2, 512], F32, name="psp", tag="ps", bufs=2)
                    if score_fp8:
                        for h in range(2):
                            nc.tensor.matmul(
                                psp[:jsz, h, vo:cw],
                                K8[ft][32 * h:32 * (h + 1), jb, :, :jsz],
                                Q8[ft][32 * h:32 * (h + 1), :,
                                       c0 + vo:c0 + cw],
                                start=True, stop=True,
                                perf_mode=mybir.MatmulPerfMode.DoubleRow,
                                tile_position=(32 * h, 0),
                            )
                    else:
                        for h in range(2):
                            d0 = D * h
                            nc.tensor.matmul(
                                psp[:jsz, h, vo:cw],
                                KTs[ft][d0:d0 + D, j0:j0 + jsz],
                                QT[ft][d0:d0 + D, c0 + vo:c0 + cw],
                                start=True, stop=True,
                                tile_position=(d0, 0),
                            )
                    if n_i == 0:
                        # boundary: the ensure_* burst was just emitted ahead
                        # of us; a little more filler hides the normalize
                        # tail and this chunk's first exp
                        pump(3, dummy=3)
                        do_norm()
                        pump(8, dummy=6)
                    else:
                        pump(2, dummy=2)
                    atp = apool.tile([P, 2, 512], MD, name="atp")
                    nc.scalar.activation(
                        out=atp[:jsz, :, vo:cw], in_=psp[:jsz, :, vo:cw],
                        func=mybir.ActivationFunctionType.Exp,
                    )
                    if j0 >= c0:
                        # zero attn where j > i on the diagonal square
                        for h in range(2):
                            nc.vector.tensor_tensor(
                                out=atp[:jsz, h, vo:vo + jsz],
                                in0=atp[:jsz, h, vo:vo + jsz],
                                in1=tri[:jsz, :jsz],
                                op=mybir.AluOpType.mult,
                            )
                    va3 = VA[jb][:].rearrange("p (h c) -> p h c", c=D + 1)
                    for h in range(2):
                        nc.tensor.matmul(
                            pso[h][:, vo:cw],
                            va3[:jsz, 2 * ft + h, :],
                            atp[:jsz, h, vo:cw],
                            start=(n_i == 0), stop=(n_i == nblk - 1),
                        )
                # row-sum copies (DVE writes to partition 0 are legal)
                ssums = []
                for h in range(2):
                    ss = spool.tile([1, 512], NRM, name=f"ssum{h}",
                                    tag=f"ssum{h}")
                    nc.vector.tensor_copy(out=ss[:, :cw], in_=pso[h][D:D + 1, :cw])
                    ssums.append(ss)

                def norm(ft=ft, ic=ic, pso=pso, ssums=ssums, c0=c0, cw=cw):
                    for h in range(2):
                        psb = psP.tile([D, 512], F32, name="psb", tag="pp",
                                       bufs=2)
                        nc.tensor.matmul(
                            psb[:, :cw], ones64[:, :], ssums[h][:, :cw],
                            start=True, stop=True,
                        )
                        rb = spool.tile([D, 512], F32, name="rb", tag="rb",
                                        bufs=2)
                        nc.vector.reciprocal_approx_fast(
                            out=rb[:, :cw], in_=psb[:, :cw])
                        nc.vector.tensor_tensor(
                            out=AOT[ft][D * h:D * (h + 1), c0:c0 + cw],
                            in0=pso[h][0:D, :cw], in1=rb[:, :cw],
                            op=mybir.AluOpType.mult,
                        )
                pending_norm[0] = norm

            # ---------------- main schedule -------------------------------
            # Startup: V blocks for chunk 0, then Q/K for ft 0 (their
            # matmuls wait on the per-kt xT DMAs, pipelining the load).
            nb0 = min(len(JB), (R_CH[0][0] + R_CH[0][1] + P - 1) // P)
            vgen = gen_v(range(len(JB)))
            ensure(vgen, lambda: v_done[0], nb0)
            projgens = [gen_projqk(ftn) for ftn in range(FT_N)]
            drain(projgens[0])
            fillers.append(vgen)
            for ftn in range(1, FT_N):
                fillers.append(projgens[ftn])

            for ft in range(FT_N):
                for ic in range(len(R_CH)):
                    # demand-driven bursts: everything this chunk's attn
                    # reads must be emitted before it, and the burst doubles
                    # as the boundary filler hiding the normalize tail
                    if ft == 0:
                        c0n, cwn = R_CH[ic]
                        nbn = min(len(JB), (c0n + cwn + P - 1) // P)
                        ensure(vgen, lambda: v_done[0], nbn)
                    if ic == 0 and 1 <= ft:
                        dma_wqk(ft + 1)
                    if ft + 1 < FT_N and ic == len(R_CH) - 1:
                        # pre-emit next ft's first proj chunk before our
                        # last chunk so the ft boundary has cover too
                        ensure(projgens[ft + 1], lambda: p_done[ft + 1], 1)
                    if ft > 0:
                        ensure(projgens[ft], lambda: p_done[ft],
                               min(ic + 2, len(R_CH)))
                    attn_chunk(ft, ic)
                    if ft == FT_N - 1:
                        # yt(ic)'s first 7 accumulations per ot only touch
                        # AOT[0..6] — pre-emit a few as cover, then the
                        # normalize, then the rest follows behind it
                        fillers.append(gen_yt(ic))
                        pump(7)
                        do_norm()
            while fillers:
                pump(10000)

    nc.compile()
    return nc


_CACHE: dict = {}


def _get_nc(causal: bool):
    if causal not in _CACHE:
        _CACHE[causal] = build2() if causal else build(causal)
    return _CACHE[causal]


def _is_causal(mask: np.ndarray) -> bool:
    if mask.shape != (S, S):
        return False
    expect = np.where(np.tril(np.ones((S, S), dtype=bool)), np.float32(0.0),
                      np.float32(NEG))
    return bool(np.array_equal(mask, expect))


MM_NP = ml_dtypes.bfloat16  # numpy dtype matching build()'s default mm_dt


def prep_inputs(x, mask, Wq, bq, Wk, Wv, bv, Wo, bo):
    """Host-side preprocessing shared by kernel() and the bench harness."""
    import ml_dtypes as mld
    scale = np.float32(1.0 / np.sqrt(D))
    xTf = np.ascontiguousarray(np.transpose(x, (0, 2, 1)).astype(np.float32))
    xT = xTf.astype(MM_NP)
    wqTf = np.ascontiguousarray((Wq.astype(np.float32) * scale).T)
    wkTf = np.ascontiguousarray(Wk.astype(np.float32).T)
    common = {
        "wqT": wqTf.astype(MM_NP),
        "wkT": wkTf.astype(MM_NP),
        "wvT": np.ascontiguousarray(Wv.astype(np.float32).T).astype(MM_NP),
        "woT": np.ascontiguousarray(Wo.astype(np.float32).T).astype(MM_NP),
        "bq": (bq.astype(np.float32) * scale),
        "bo": (bo.astype(np.float32) + Wo.astype(np.float32) @ bv.astype(np.float32)),
        # fp8 DoubleRow projection operands (scaled to dodge e4m3 denormals)
        "wq8": np.ascontiguousarray(
            (wqTf * W8SCALE).reshape(8, P, E).transpose(1, 0, 2)
        ).astype(mld.float8_e4m3fn),
        "wk8": np.ascontiguousarray(
            (wkTf * W8SCALE).reshape(8, P, E).transpose(1, 0, 2)
        ).astype(mld.float8_e4m3fn),
    }
    x8 = np.ascontiguousarray(
        xTf.reshape(B, 8, P, S).transpose(0, 2, 1, 3)
    ).astype(mld.float8_e4m3fn)
    causal = _is_causal(np.asarray(mask))
    if not causal:
        common["maskT"] = np.ascontiguousarray(np.asarray(mask, np.float32).T)
    in_maps = [dict(common, xT=xT[b], x8=x8[b]) for b in range(B)]
    return causal, in_maps


_RUNNER: dict = {}


def _get_runner(causal: bool):
    """Compile once per mask-variant; cache the jitted SPMD executable."""
    if causal in _RUNNER:
        return _RUNNER[causal]
    import jax
    from jax.sharding import Mesh, PartitionSpec, NamedSharding
    import warnings
    with warnings.catch_warnings():
        warnings.simplefilter("ignore")
        from jax.experimental.shard_map import shard_map
    from concourse import bass2jax
    from concourse.bass2jax import _bass_exec_p, install_neuronx_cc_hook

    nc = _get_nc(causal)
    install_neuronx_cc_hook()
    partition_name = (nc.partition_id_tensor.name
                      if nc.partition_id_tensor else None)
    in_names, out_names, out_avals = [], [], []
    for alloc in nc.m.functions[0].allocations:
        if not isinstance(alloc, mybir.MemoryLocationSet):
            continue
        name = alloc.memorylocations[0].name
        if alloc.kind == "ExternalInput":
            if name != partition_name:
                in_names.append(name)
        elif alloc.kind == "ExternalOutput":
            out_names.append(name)
            out_avals.append(jax.core.ShapedArray(
                tuple(alloc.tensor_shape), mybir.dt.np(alloc.dtype)))
    n_params = len(in_names)
    n_outs = len(out_names)

    def _body(*args):
        operands = list(args)
        names = list(in_names) + list(out_names)
        if partition_name is not None:
            operands.append(bass2jax.partition_id_tensor())
            names.append(partition_name)
        outs = _bass_exec_p.bind(
            *operands,
            out_avals=tuple(out_avals),
            in_names=tuple(names),
            out_names=tuple(out_names),
            lowering_input_output_aliases=(),
            sim_require_finite=True,
            sim_require_nnan=True,
            nc=nc,
        )
        return tuple(outs)

    devices = jax.devices()[:B]
    mesh = Mesh(np.asarray(devices), ("core",))
    in_specs = (PartitionSpec("core"),) * (n_params + n_outs)
    out_specs = (PartitionSpec("core"),) * n_outs
    fn = jax.jit(
        shard_map(_body, mesh=mesh, in_specs=in_specs, out_specs=out_specs,
                  check_rep=False),
        donate_argnums=tuple(range(n_params, n_params + n_outs)),
        keep_unused=True,
    )
    runner = (fn, in_names, out_names, out_avals)
    _RUNNER[causal] = runner
    return runner


def kernel(x, mask, Wq, bq, Wk, Wv, bv, Wo, bo):
    causal, in_maps = prep_inputs(x, mask, Wq, bq, Wk, Wv, bv, Wo, bo)
    fn, in_names, out_names, out_avals = _get_runner(causal)
    cat = [np.concatenate([np.asarray(m[n]) for m in in_maps], axis=0)
           for n in in_names]
    zs = [np.zeros((B * a.shape[0], *a.shape[1:]), a.dtype) for a in out_avals]
    outs = fn(*cat, *zs)
    yT = np.asarray(outs[out_names.index("yT")]).reshape(B, E, S)
    out = np.ascontiguousarray(yT.transpose(0, 2, 1).astype(np.float32))
    return out



# revision 51
# speedup vs baseline: 1.2288x; 1.0410x over previous
"""Multi-head attention (B=8, S=1500, E=1024, H=16, D=64) on 8 trn2 NeuronCores.

Sharding: pure data-parallel over batch — core b computes batch element b
end-to-end (no collectives). Host pre-transposes x and the weights so every
device-side matmul has its contraction dim on the SBUF partition axis, and
folds the 1/sqrt(D) scale into Wq/bq and the V-bias into the output bias
(bo_eff = bo + Wo @ bv), so the device kernel never touches bv.

Device pipeline per core (all f32 storage, matmuls run as float32r):
  QT = (Wq/8)^T-proj of x^T   [1024, 1500]  (f-on-partition; bias bq/8 per-partition)
  KT = Wk^T-proj              [1024, 1500]
  V_aug = x @ Wv^T with a ones-column appended per head  [1500, 16*65]
  per (i-chunk, head): scoresT[j, i] via matmul(lhsT=KT_h, rhs=QT_h);
    exp on ACT (no max-subtraction: |scores| <~ 30, safely inside f32);
    causal masking structurally (affine_select zero-fill on diagonal blocks)
    or via an additive mask tensor (general path);
  out^T + rowsums in ONE matmul: lhsT=[V_h | 1] (65 cols), rhs=attnT;
  normalize: recip of the sums row, rank-1 matmul broadcast across 64
    partitions, multiply on eviction -> AO^T;
  yT = Wo^T-proj of AO^T + bo_eff  -> DRAM [1024, 1500], host transposes back.
"""

import sys
import numpy as np
import ml_dtypes

for _p in ("/opt/trn_rl_repo",):
    if _p not in sys.path:
        sys.path.append(_p)

import concourse.bass as bass
import concourse.mybir as mybir
import concourse.tile as tile
from concourse import bacc
from concourse.bass_utils import run_bass_kernel_spmd

F32 = mybir.dt.float32

B, S, E, H, D = 8, 1500, 1024, 16, 64
P = 128
NEG = -1e9


def _chunks(total, step):
    return [(c0, min(step, total - c0)) for c0 in range(0, total, step)]


def _wslices(dram_ap, col0, cols):
    """[E, E] weight -> [P, E//P, cols] AP for a column slice (k on partition)."""
    return dram_ap.rearrange("(kt p) f -> p kt f", p=P)[:, :, col0:col0 + cols]


def build(causal: bool, mm_dt=mybir.dt.bfloat16):
    KT_N = E // P            # k-tiles over the embedding dim
    FT_N = E // P            # f-tiles
    R_CH = _chunks(S, 512)   # i/r chunks
    JB = _chunks(S, P)       # j blocks
    H_LOC = E // D
    nc = bacc.Bacc("TRN2", target_bir_lowering=False, debug=False, num_devices=8)
    MD = mm_dt  # dtype for every matmul operand chain
    NRM = mybir.dt.float32r if MD == mybir.dt.bfloat16 else MD

    xT = nc.dram_tensor("xT", [E, S], MD, kind="ExternalInput")
    wqT = nc.dram_tensor("wqT", [E, E], MD, kind="ExternalInput")
    wkT = nc.dram_tensor("wkT", [E, E], MD, kind="ExternalInput")
    wvT = nc.dram_tensor("wvT", [E, E], MD, kind="ExternalInput")
    woT = nc.dram_tensor("woT", [E, E], MD, kind="ExternalInput")
    bq = nc.dram_tensor("bq", [E], F32, kind="ExternalInput")
    bo = nc.dram_tensor("bo", [E], F32, kind="ExternalInput")
    maskT = None
    if not causal:
        maskT = nc.dram_tensor("maskT", [S, S], F32, kind="ExternalInput")
    yT = nc.dram_tensor("yT", [E, S], F32, kind="ExternalOutput")

    def mm(ap):
        return ap

    nc._allow_low_precision_reason = "low-precision matmul operand chain"
    with tile.TileContext(nc) as tc:
        with (
            tc.tile_pool(name="persist", bufs=1) as pers,
            tc.tile_pool(name="wqkp", bufs=2) as wqkp,
            tc.tile_pool(name="wvp", bufs=1) as wvp,
            tc.tile_pool(name="wop", bufs=1) as wop,
            tc.tile_pool(name="attn", bufs=3) as apool,
            tc.tile_pool(name="small", bufs=3) as spool,
            tc.tile_pool(name="evp", bufs=3) as evp,
            tc.tile_pool(name="psP", bufs=1, space="PSUM") as psP,
            tc.tile_pool(name="psS", bufs=1, space="PSUM") as psS,
            tc.tile_pool(name="psO", bufs=1, space="PSUM") as psO,
            tc.tile_pool(name="psY", bufs=1, space="PSUM") as psY,
        ):
            ones64 = pers.tile([1, D], NRM, name="ones64")
            nc.vector.memset(ones64[:].bitcast(F32), 1.0)
            bq_sb = pers.tile([P, FT_N], F32, name="bq_sb")
            nc.sync.dma_start(out=bq_sb[:], in_=bq.ap().rearrange("(t p) -> p t", p=P))
            bo_sb = pers.tile([P, FT_N], F32, name="bo_sb")
            nc.sync.dma_start(out=bo_sb[:], in_=bo.ap().rearrange("(t p) -> p t", p=P))

            # upper-triangular (incl diag) 0/1 mask for diagonal attn blocks
            tri32 = pers.tile([P, P], F32, name="tri32")
            nc.gpsimd.memset(tri32[:], 1.0)
            nc.gpsimd.affine_select(
                out=tri32[:], in_=tri32[:],
                pattern=[[1, P]], compare_op=mybir.AluOpType.is_ge,
                fill=0.0, base=0, channel_multiplier=-1,
            )
            tri = pers.tile([P, P], MD, name="tri")
            nc.vector.tensor_copy(out=tri[:], in_=tri32[:])

            XT = [pers.tile([P, S], MD, name=f"xt{kt}") for kt in range(KT_N)]
            QT = [pers.tile([P, S], MD, name=f"qt{ft}") for ft in range(FT_N)]
            KTs = [pers.tile([P, S], MD, name=f"kt{ft}") for ft in range(FT_N)]
            VA = [pers.tile([P, H_LOC * (D + 1)], MD, name=f"va{rt}")
                  for rt in range(len(JB))]
            AOT = [pers.tile([P, S], MD, name=f"aot{ft}") for ft in range(FT_N)]

            # ---- V projection (natural layout, ones column appended) ----
            hpc = 512 // D  # heads per 512-wide f chunk
            fchunks = _chunks(E, 512)
            wv_tiles = [wvp.tile([P, KT_N, 512], MD, name=f"wv{fc}", tag=f"wv{fc}")
                        for fc in range(len(fchunks))]
            # interleave wv-slice and xT-tile loads so the first V matmuls
            # can issue as soon as (wv[:,0,:], xT[0]) land
            for kt in range(KT_N):
                for fc, (f0, fw) in enumerate(fchunks):
                    nc.sync.dma_start(
                        out=wv_tiles[fc][:, kt, :fw],
                        in_=_wslices(wvT.ap(), f0, fw)[:, kt, :])
                nc.sync.dma_start(out=XT[kt][:],
                                  in_=xT[kt * P:(kt + 1) * P, :])

            def emit_v(rts):
                for rt in rts:
                    if rt >= len(JB):
                        continue
                    r0, rsz = JB[rt]
                    for fc, (f0, fw) in enumerate(fchunks):
                        wt = wv_tiles[fc]
                        ps = psP.tile([P, 512], F32, name="pv", tag="pp", bufs=2)
                        for kt in range(KT_N):
                            nc.tensor.matmul(
                                ps[:rsz, :fw],
                                mm(XT[kt][:, r0:r0 + rsz]),
                                mm(wt[:, kt, :fw]),
                                start=(kt == 0), stop=(kt == KT_N - 1),
                            )
                        dst = VA[rt][:].rearrange("p (h c) -> p h c", c=D + 1)
                        nc.vector.tensor_copy(
                            out=dst[:rsz, fc * hpc:fc * hpc + fw // D, 0:D],
                            in_=ps[:rsz, :fw].rearrange("p (h d) -> p h d", d=D),
                        )
                    va3 = VA[rt][:].rearrange("p (h c) -> p h c", c=D + 1)
                    if MD == mybir.dt.float32r:
                        nc.gpsimd.memset(va3[:rsz, :, D:D + 1].bitcast(F32), 1.0)
                    else:
                        nc.gpsimd.memset(va3[:rsz, :, D:D + 1], 1.0)

            def proj_qk_gen(ft):
                for which, wdram, dst in (("q", wqT, QT), ("k", wkT, KTs)):
                    wt = wqkp.tile([P, KT_N, P], MD, name="wqk", tag="wqk")
                    nc.sync.dma_start(out=wt[:], in_=_wslices(wdram.ap(), ft * P, P))
                    for rc, (c0, cw) in enumerate(R_CH):
                        ps = psP.tile([P, 512], F32, name="pp", tag="pp", bufs=2)
                        for kt in range(KT_N):
                            nc.tensor.matmul(
                                ps[:, :cw],
                                mm(wt[:, kt, :]),
                                mm(XT[kt][:, c0:c0 + cw]),
                                start=(kt == 0), stop=(kt == KT_N - 1),
                            )
                        if which == "q":
                            nc.vector.tensor_scalar(
                                out=dst[ft][:, c0:c0 + cw], in0=ps[:, :cw],
                                scalar1=bq_sb[:, ft:ft + 1], scalar2=None,
                                op0=mybir.AluOpType.add,
                            )
                        else:
                            nc.vector.tensor_copy(
                                out=dst[ft][:, c0:c0 + cw], in_=ps[:, :cw])
                        yield

            def proj_qk(ft):
                for _ in proj_qk_gen(ft):
                    pass

            def attn_ft(ic, ft, mtiles, filler=None):
                c0, cw = R_CH[ic]
                nblk = (min(len(JB), (c0 + cw + P - 1) // P)
                        if causal else len(JB))
                pso = [psO.tile([D + 1, 512], F32, name=f"po{half}",
                                tag="po", bufs=2)
                       for half in range(2)]
                # diagonal-containing blocks first so the chunk-end attnV
                # gates on a short (non-masked) exp chain
                if causal:
                    cut = max(0, nblk - (cw + P - 1) // P)
                    order = list(range(cut, nblk)) + list(range(cut))
                else:
                    order = list(range(nblk))
                for n_i, jb in enumerate(order):
                    j0, jsz = JB[jb]
                    vo = max(0, j0 - c0) if causal else 0
                    # both halves' scores land in one 2-bank psum pair so a
                    # single ACTIVATE exps them together (halves ACT op count)
                    psp = psS.tile([P, 2, 512], F32, name="psp",
                                   tag="ps", bufs=2)
                    for half in range(2):
                        d0 = D * half
                        nc.tensor.matmul(
                            psp[:jsz, half, vo:cw],
                            mm(KTs[ft][d0:d0 + D, j0:j0 + jsz]),
                            mm(QT[ft][d0:d0 + D, c0 + vo:c0 + cw]),
                            start=True, stop=True,
                            tile_position=(d0, 0),
                        )
                    if not causal:
                        for half in range(2):
                            nc.vector.tensor_tensor(
                                out=psp[:jsz, half, :cw],
                                in0=psp[:jsz, half, :cw],
                                in1=mtiles[jb][:jsz, :cw],
                                op=mybir.AluOpType.add,
                            )
                    atp = apool.tile([P, 2, 512], MD, name="atp")
                    nc.scalar.activation(
                        out=atp[:jsz, :, vo:cw], in_=psp[:jsz, :, vo:cw],
                        func=mybir.ActivationFunctionType.Exp,
                    )
                    if causal and j0 >= c0:
                        # zero attn where j > i on the diagonal square
                        for half in range(2):
                            nc.vector.tensor_tensor(
                                out=atp[:jsz, half, vo:vo + jsz],
                                in0=atp[:jsz, half, vo:vo + jsz],
                                in1=tri[:jsz, :jsz],
                                op=mybir.AluOpType.mult,
                            )
                    va3 = VA[jb][:].rearrange("p (h c) -> p h c", c=D + 1)
                    for half in range(2):
                        nc.tensor.matmul(
                            pso[half][:, vo:cw],
                            mm(va3[:jsz, 2 * ft + half, :]),
                            mm(atp[:jsz, half, vo:cw]),
                            start=(n_i == 0), stop=(n_i == nblk - 1),
                        )
                    if filler is not None and n_i % 4 == 3:
                        filler()
                ssums = []
                for half in range(2):
                    ssum = spool.tile([1, 512], NRM, name=f"ssum{half}",
                                      tag="ssum")
                    nc.vector.tensor_copy(
                        out=ssum[:, :cw], in_=pso[half][D:D + 1, :cw])
                    ssums.append(ssum)
                for half in range(2):
                    d0 = D * half
                    psb = psP.tile([D, 512], F32, name="psb", tag="pp", bufs=2)
                    nc.tensor.matmul(
                        psb[:, :cw], mm(ones64[:, :]), mm(ssums[half][:, :cw]),
                        start=True, stop=True,
                    )
                    rb = spool.tile([D, 512], F32, name="rb")
                    nc.vector.reciprocal_approx_fast(
                        out=rb[:, :cw], in_=psb[:, :cw])
                    nc.vector.tensor_tensor(
                        out=AOT[ft][d0:d0 + D, c0:c0 + cw],
                        in0=pso[half][0:D, :cw], in1=rb[:, :cw],
                        op=mybir.AluOpType.mult,
                    )

            def emit_yt(ot, rc, wo_t):
                c0, cw = R_CH[rc]
                psy = psP.tile([P, 512], F32, name="py", tag="pp", bufs=2)
                for ft in range(FT_N):
                    nc.tensor.matmul(
                        psy[:, :cw],
                        mm(wo_t[:, ft, :]),
                        mm(AOT[ft][:, c0:c0 + cw]),
                        start=(ft == 0), stop=(ft == FT_N - 1),
                    )
                yt = evp.tile([P, 512], F32, name="yt", tag="yt")
                nc.vector.tensor_scalar(
                    out=yt[:, :cw], in0=psy[:, :cw],
                    scalar1=bo_sb[:, ot:ot + 1], scalar2=None,
                    op0=mybir.AluOpType.add,
                )
                nc.sync.dma_start(
                    out=yT[ot * P:(ot + 1) * P, c0:c0 + cw], in_=yt[:, :cw])

            if causal:
                wo_tiles = []
                for ot in range(FT_N):
                    wt = wop.tile([P, KT_N, P], MD, name=f"wo{ot}",
                                  tag=f"wo{ot}")
                    nc.sync.dma_start(out=wt[:],
                                      in_=_wslices(woT.ap(), ot * P, P))
                    wo_tiles.append(wt)
                nb0 = min(len(JB), (R_CH[0][0] + R_CH[0][1] + P - 1) // P)
                emit_v(range(nb0))
                proj_qk(0)
                nbp = nb0
                for ft in range(FT_N):
                    gen = proj_qk_gen(ft + 1) if ft + 1 < FT_N else None

                    def pump():
                        if gen is not None:
                            next(gen, None)

                    for ic in range(len(R_CH)):
                        attn_ft(ic, ft, None, filler=pump)
                        if ft == 0 and ic + 1 < len(R_CH):
                            c0n, cwn = R_CH[ic + 1]
                            nbn = min(len(JB), (c0n + cwn + P - 1) // P)
                            emit_v(range(nbp, nbn))
                            nbp = nbn
                        if ft == FT_N - 1:
                            # last ft has no next-ft projection filler: use the
                            # now-ready yT chunk as PE filler instead
                            for ot in range(FT_N):
                                emit_yt(ot, ic, wo_tiles[ot])
                    if gen is not None:
                        for _ in gen:
                            pass
            else:
                emit_v(range(len(JB)))
                for ft in range(FT_N):
                    proj_qk(ft)
                with tc.tile_pool(name="maskp", bufs=1) as mpool:
                    for ic, (c0, cw) in enumerate(R_CH):
                        mtiles = []
                        for jb, (j0, jsz) in enumerate(JB):
                            mt = mpool.tile([P, 512], F32, name=f"m{jb}")
                            nc.sync.dma_start(
                                out=mt[:jsz, :cw],
                                in_=maskT[j0:j0 + jsz, c0:c0 + cw])
                            mtiles.append(mt)
                        for ft in range(FT_N):
                            attn_ft(ic, ft, mtiles)
                for ot in range(FT_N):
                    wt = wop.tile([P, KT_N, P], MD, name=f"wo{ot}", tag="wo",
                                  bufs=2)
                    nc.sync.dma_start(out=wt[:], in_=_wslices(woT.ap(), ot * P, P))
                    for rc in range(len(R_CH)):
                        emit_yt(ot, rc, wt)

    nc.compile()
    return nc


W8SCALE = 256.0  # host-side fp8 weight scale (undone at psum eviction)


def build2(score_fp8: bool = False, proj_fp8: bool = True,
           mm_dt=mybir.dt.bfloat16):
    """Causal-only v2: gap-free PE schedule + optional fp8 DoubleRow scores.

    Emission-order discipline: the PE queue is in-order, so every
    potentially-waiting matmul has independent filler work (next-ft Q/K
    projection, V-projection, or output-projection matmuls) queued ahead
    of it.  Fillers are drained from a FIFO of generators, with a burst
    at every (ft, chunk) boundary so the softmax-normalize tail and the
    next chunk's first exp are fully hidden.  PE gaps also reset the PE
    clock ramp (0.65/1.2/2.4 GHz p-states), so continuity matters twice.

    Normalize uses one K=2 rank-2 matmul per (ft, chunk) broadcasting
    both halves' row-sums (f32r), with the reciprocal on a [128, cw]
    tile.  Diagonal-block causal masking runs on the idle Pool engine
    via affine_select; row-sum copies also go to Pool.

    score_fp8: Q/K are evicted to fp8e4 in DoubleRow-packed layout
    ([32 partitions, 2 k-planes]) and scores run as fp8 DoubleRow
    matmuls at 0.5 cycles/column.
    """
    KT_N = E // P
    FT_N = E // P
    R_CH = _chunks(S, 512)
    JB = _chunks(S, P)
    H_LOC = E // D
    nc = bacc.Bacc("TRN2", target_bir_lowering=False, debug=False, num_devices=8)
    MD = mm_dt
    F8 = mybir.dt.float8e4
    NRM = mybir.dt.float32r

    xT = nc.dram_tensor("xT", [E, S], MD, kind="ExternalInput")
    if proj_fp8:
        # DoubleRow fp8 Q/K projections: x8[p, kt, i] = x[kt*128+p, i],
        # w{q,k}8[p, kt, f] = W^T[kt*128+p, f] * W8SCALE
        x8d = nc.dram_tensor("x8", [P, KT_N, S], F8, kind="ExternalInput")
        wq8d = nc.dram_tensor("wq8", [P, KT_N, E], F8, kind="ExternalInput")
        wk8d = nc.dram_tensor("wk8", [P, KT_N, E], F8, kind="ExternalInput")
    else:
        wqT = nc.dram_tensor("wqT", [E, E], MD, kind="ExternalInput")
        wkT = nc.dram_tensor("wkT", [E, E], MD, kind="ExternalInput")
    wvT = nc.dram_tensor("wvT", [E, E], MD, kind="ExternalInput")
    woT = nc.dram_tensor("woT", [E, E], MD, kind="ExternalInput")
    bq = nc.dram_tensor("bq", [E], F32, kind="ExternalInput")
    bo = nc.dram_tensor("bo", [E], F32, kind="ExternalInput")
    yT = nc.dram_tensor("yT", [E, S], F32, kind="ExternalOutput")

    nc._allow_low_precision_reason = "low-precision matmul operand chain"
    with tile.TileContext(nc) as tc:
        with (
            tc.tile_pool(name="persist", bufs=1) as pers,
            tc.tile_pool(name="wpool", bufs=1) as wpool,
            tc.tile_pool(name="attn", bufs=3) as apool,
            tc.tile_pool(name="small", bufs=2) as spool,
            tc.tile_pool(name="evp", bufs=3) as evp,
            tc.tile_pool(name="psP", bufs=1, space="PSUM") as psP,
            tc.tile_pool(name="psS", bufs=1, space="PSUM") as psS,
            tc.tile_pool(name="psO", bufs=1, space="PSUM") as psO,
        ):
            # ---- persistent small tiles / DMA preload (priority order) ----
            bq_sb = pers.tile([P, FT_N], F32, name="bq_sb")
            nc.sync.dma_start(out=bq_sb[:], in_=bq.ap().rearrange("(t p) -> p t", p=P))
            bo_sb = pers.tile([P, FT_N], F32, name="bo_sb")
            nc.sync.dma_start(out=bo_sb[:], in_=bo.ap().rearrange("(t p) -> p t", p=P))

            # Q/K weights streamed per-ft (4 rotating slots, DMA issued
            # ~2 chunks ahead of first use)
            wqk_t: dict = {}
            W8DT = F8 if proj_fp8 else MD

            def dma_wqk(ft):
                if ft in wqk_t or ft >= FT_N:
                    return
                tq = wpool.tile([P, KT_N, P], W8DT, name=f"wq{ft}")
                tk = wpool.tile([P, KT_N, P], W8DT, name=f"wk{ft}")
                if proj_fp8:
                    nc.sync.dma_start(out=tq[:],
                                      in_=wq8d[:, :, ft * P:(ft + 1) * P])
                    nc.sync.dma_start(out=tk[:],
                                      in_=wk8d[:, :, ft * P:(ft + 1) * P])
                else:
                    nc.sync.dma_start(out=tq[:], in_=_wslices(wqT.ap(), ft * P, P))
                    nc.sync.dma_start(out=tk[:], in_=_wslices(wkT.ap(), ft * P, P))
                wqk_t[ft] = (tq, tk)

            dma_wqk(0)

            # xT and wv interleaved per-kt (startup critical path)
            XT = [pers.tile([P, S], MD, name=f"xt{kt}") for kt in range(KT_N)]
            fchunks = _chunks(E, 512)
            hpc = 512 // D
            wv_tiles = [wpool.tile([P, KT_N, 512], MD, name=f"wv{fc}")
                        for fc in range(len(fchunks))]
            X8 = None
            if proj_fp8:
                X8 = pers.tile([P, KT_N, S], F8, name="x8sb")
            for kt in range(KT_N):
                nc.sync.dma_start(out=XT[kt][:], in_=xT[kt * P:(kt + 1) * P, :])
                if proj_fp8:
                    nc.sync.dma_start(out=X8[:, kt, :], in_=x8d[:, kt, :])
                for fc, (f0, fw) in enumerate(fchunks):
                    nc.sync.dma_start(
                        out=wv_tiles[fc][:, kt, :fw],
                        in_=_wslices(wvT.ap(), f0, fw)[:, kt, :])
            for _ft in range(1, FT_N):
                dma_wqk(_ft)
            # output-projection weights last
            wo_tiles = [wpool.tile([P, KT_N, P], MD, name=f"wo{ot}")
                        for ot in range(FT_N)]
            for ot in range(FT_N):
                nc.sync.dma_start(out=wo_tiles[ot][:], in_=_wslices(woT.ap(), ot * P, P))

            ones64 = pers.tile([1, D], NRM, name="ones64")
            nc.vector.memset(ones64[:].bitcast(F32), 1.0)

            # upper-triangular (incl diag) 0/1 mask for diagonal attn blocks
            tri32 = pers.tile([P, P], F32, name="tri32")
            nc.gpsimd.memset(tri32[:], 1.0)
            nc.gpsimd.affine_select(
                out=tri32[:], in_=tri32[:],
                pattern=[[1, P]], compare_op=mybir.AluOpType.is_ge,
                fill=0.0, base=0, channel_multiplier=-1,
            )
            tri = pers.tile([P, P], MD, name="tri")
            nc.vector.tensor_copy(out=tri[:], in_=tri32[:])

            VA = [pers.tile([P, H_LOC * (D + 1)], MD, name=f"va{rt}")
                  for rt in range(len(JB))]
            AOT = [pers.tile([P, S], MD, name=f"aot{ft}") for ft in range(FT_N)]
            if score_fp8:
                # DoubleRow-packed Q/K per ft, fp8.  Partitions [32h:32h+32)
                # hold head h's 64 dims as 2 k-planes (d = 32*plane + p).
                # Q is the moving operand (no stride restriction): [2, S].
                # K is the stationary operand — LDWEIGHTS dual-fp8 requires a
                # power-of-two plane stride, so K is blocked per j-block of
                # 128 with padding: [12, 2, 128].
                Q8 = [pers.tile([P, 2, S], F8, name=f"q8_{ft}")
                      for ft in range(FT_N)]
                K8 = [pers.tile([P, len(JB), 2, P], F8, name=f"k8_{ft}")
                      for ft in range(FT_N)]
            else:
                QT = [pers.tile([P, S], MD, name=f"qt{ft}") for ft in range(FT_N)]
                KTs = [pers.tile([P, S], MD, name=f"kt{ft}") for ft in range(FT_N)]

            # ---------------- work generators (PE-quantum yields) --------
            v_done = [0]  # highest V block index fully emitted + 1

            def gen_v(rts):
                for rt in rts:
                    r0, rsz = JB[rt]
                    for fc, (f0, fw) in enumerate(fchunks):
                        ps = psP.tile([P, 512], F32, name="pv", tag="pp", bufs=2)
                        for kt in range(KT_N):
                            nc.tensor.matmul(
                                ps[:rsz, :fw],
                                XT[kt][:, r0:r0 + rsz],
                                wv_tiles[fc][:, kt, :fw],
                                start=(kt == 0), stop=(kt == KT_N - 1),
                            )
                            yield
                        dst = VA[rt][:].rearrange("p (h c) -> p h c", c=D + 1)
                        nc.vector.tensor_copy(
                            out=dst[:rsz, fc * hpc:fc * hpc + fw // D, 0:D],
                            in_=ps[:rsz, :fw].rearrange("p (h d) -> p h d", d=D),
                        )
                    va3 = VA[rt][:].rearrange("p (h c) -> p h c", c=D + 1)
                    nc.gpsimd.memset(va3[:rsz, :, D:D + 1], 1.0)
                    v_done[0] = rt + 1

            pc_done: set = set()  # (ft, rc) proj chunks fully emitted

            def gen_proj_ic(rc):
                c0, cw = R_CH[rc]
                for ft in range(FT_N):
                    dma_wqk(ft)  # idempotent; normally prefetched earlier
                    for which in range(2):  # 0=q, 1=k
                        wt = wqk_t[ft][which]
                        ps = psP.tile([P, 512], F32, name="pp", tag="pp", bufs=2)
                        if proj_fp8:
                            # DoubleRow: one instruction contracts 2 k-tiles
                            # (256 rows); split columns in half for finer
                            # filler quanta
                            nch = (cw + 255) // 256
                            for ch in range(nch):
                                cl = ch * 256
                                chw = min(256, cw - cl)
                                for t in range(KT_N // 2):
                                    nc.tensor.matmul(
                                        ps[:, cl:cl + chw],
                                        wt[:, 2 * t:2 * t + 2, :],
                                        X8[:, 2 * t:2 * t + 2,
                                           c0 + cl:c0 + cl + chw],
                                        start=(t == 0),
                                        stop=(t == KT_N // 2 - 1),
                                        perf_mode=mybir.MatmulPerfMode.DoubleRow,
                                    )
                                    yield
                        else:
                            for kt in range(KT_N):
                                nc.tensor.matmul(
                                    ps[:, :cw],
                                    wt[:, kt, :],
                                    XT[kt][:, c0:c0 + cw],
                                    start=(kt == 0), stop=(kt == KT_N - 1),
                                )
                                yield
                        inv = 1.0 / W8SCALE if proj_fp8 else None
                        if score_fp8:
                            stg = spool.tile([P, 512], F8, name="stg", tag="stg",
                                             bufs=2)
                            if which == 0:
                                nc.vector.tensor_scalar(
                                    out=stg[:, :cw], in0=ps[:, :cw],
                                    scalar1=inv if inv else 1.0,
                                    scalar2=bq_sb[:, ft:ft + 1],
                                    op0=mybir.AluOpType.mult,
                                    op1=mybir.AluOpType.add,
                                )
                            else:
                                nc.vector.tensor_scalar(
                                    out=stg[:, :cw], in0=ps[:, :cw],
                                    scalar1=inv if inv else 1.0, scalar2=None,
                                    op0=mybir.AluOpType.mult,
                                )
                            for h in range(2):
                                for j in range(2):
                                    src = stg[64 * h + 32 * j:
                                              64 * h + 32 * (j + 1), :cw]
                                    if which == 0:
                                        nc.sync.dma_start(
                                            out=Q8[ft][32 * h:32 * (h + 1),
                                                       j, c0:c0 + cw],
                                            in_=src)
                                    else:
                                        # K: into 128-col padded blocks
                                        b0 = 4 * rc
                                        nfull = cw // P
                                        nc.sync.dma_start(
                                            out=K8[ft][32 * h:32 * (h + 1),
                                                       b0:b0 + nfull, j, :],
                                            in_=src[:, :nfull * P].rearrange(
                                                "p (b c) -> p b c", c=P))
                                        if cw % P:
                                            nc.sync.dma_start(
                                                out=K8[ft][32 * h:32 * (h + 1),
                                                           b0 + nfull, j,
                                                           :cw % P],
                                                in_=src[:, nfull * P:cw])
                        else:
                            dst = QT if which == 0 else KTs
                            if which == 0:
                                if inv is not None:
                                    nc.vector.tensor_scalar(
                                        out=dst[ft][:, c0:c0 + cw],
                                        in0=ps[:, :cw],
                                        scalar1=inv,
                                        scalar2=bq_sb[:, ft:ft + 1],
                                        op0=mybir.AluOpType.mult,
                                        op1=mybir.AluOpType.add,
                                    )
                                else:
                                    nc.vector.tensor_scalar(
                                        out=dst[ft][:, c0:c0 + cw],
                                        in0=ps[:, :cw],
                                        scalar1=bq_sb[:, ft:ft + 1],
                                        scalar2=None,
                                        op0=mybir.AluOpType.add,
                                    )
                            elif inv is not None:
                                nc.vector.tensor_scalar(
                                    out=dst[ft][:, c0:c0 + cw], in0=ps[:, :cw],
                                    scalar1=inv, scalar2=None,
                                    op0=mybir.AluOpType.mult,
                                )
                            else:
                                nc.vector.tensor_copy(
                                    out=dst[ft][:, c0:c0 + cw], in_=ps[:, :cw])
                    pc_done.add((ft, rc))

            def gen_yt(rc):
                c0, cw = R_CH[rc]
                for ot in range(FT_N):
                    psy = psP.tile([P, 512], F32, name="py", tag="pp", bufs=2)
                    for ft in range(FT_N):
                        nc.tensor.matmul(
                            psy[:, :cw],
                            wo_tiles[ot][:, ft, :],
                            AOT[ft][:, c0:c0 + cw],
                            start=(ft == 0), stop=(ft == FT_N - 1),
                        )
                        yield
                    ytt = evp.tile([P, 512], F32, name="yt", tag="yt")
                    nc.vector.tensor_scalar(
                        out=ytt[:, :cw], in0=psy[:, :cw],
                        scalar1=bo_sb[:, ot:ot + 1], scalar2=None,
                        op0=mybir.AluOpType.add,
                    )
                    nc.sync.dma_start(
                        out=yT[ot * P:(ot + 1) * P, c0:c0 + cw], in_=ytt[:, :cw])

            # ---------------- filler FIFO ---------------------------------
            fillers: list = []

            def pump(n=1, dummy=0):
                # pull up to n real filler quanta; if the FIFO runs dry,
                # emit up to `dummy` no-op LDWEIGHTS — they keep the PE
                # executing (clock p-state) across dependency waits and are
                # harmless since every matmul reloads its own weights
                while n > 0 and fillers:
                    try:
                        next(fillers[0])
                        n -= 1
                    except StopIteration:
                        fillers.pop(0)
                for _ in range(min(n, dummy)):
                    nc.tensor.ldweights(tri[:, :])

            def drain(g):
                if g is None:
                    return
                for _ in g:
                    pass
                if fillers and fillers[0] is g:
                    fillers.pop(0)

            def ensure(g, get, target):
                while get() < target:
                    try:
                        next(g)
                    except StopIteration:
                        break

            # ---------------- attention -----------------------------------
            pending_norm = [None]

            def do_norm():
                fn = pending_norm[0]
                pending_norm[0] = None
                if fn is not None:
                    fn()

            def attn_chunk(ft, ic):
                c0, cw = R_CH[ic]
                nblk = min(len(JB), (c0 + cw + P - 1) // P)
                # natural order = diagonal blocks last, so their extra
                # mask-latency lands at the chunk end where the deferred
                # normalize + next boundary burst hide it
                order = list(range(nblk))
                pso = [psO.tile([D + 1, 512], F32, name=f"po{h}", tag="po", bufs=2)
                       for h in range(2)]
                for n_i, jb in enumerate(order):
                    j0, jsz = JB[jb]
                    vo = max(0, j0 - c0)
                    cwv = cw - vo
                    psp = psS.tile([P, 2, 512], F32, name="psp", tag="ps", bufs=2)
                    if score_fp8:
                        for h in range(2):
                            nc.tensor.matmul(
                                psp[:jsz, h, vo:cw],
                                K8[ft][32 * h:32 * (h + 1), jb, :, :jsz],
                                Q8[ft][32 * h:32 * (h + 1), :,
                                       c0 + vo:c0 + cw],
                                start=True, stop=True,
                                perf_mode=mybir.MatmulPerfMode.DoubleRow,
                                tile_position=(32 * h, 0),
                            )
                    else:
                        for h in range(2):
                            d0 = D * h
                            nc.tensor.matmul(
                                psp[:jsz, h, vo:cw],
                                KTs[ft][d0:d0 + D, j0:j0 + jsz],
                                QT[ft][d0:d0 + D, c0 + vo:c0 + cw],
                                start=True, stop=True,
                                tile_position=(d0, 0),
                            )
                    if n_i == 0:
                        # boundary: the ensure_* burst was just emitted ahead
                        # of us; a little more filler hides the normalize
                        # tail and this chunk's first exp
                        pump(3, dummy=3)
                        do_norm()
                        pump(8, dummy=6)
                    else:
                        pump(2, dummy=2)
                    atp = apool.tile([P, 2, 512], MD, name="atp")
                    nc.scalar.activation(
                        out=atp[:jsz, :, vo:cw], in_=psp[:jsz, :, vo:cw],
                        func=mybir.ActivationFunctionType.Exp,
                    )
                    if j0 >= c0:
                        # zero attn where j > i on the diagonal square
                        for h in range(2):
                            nc.vector.tensor_tensor(
                                out=atp[:jsz, h, vo:vo + jsz],
                                in0=atp[:jsz, h, vo:vo + jsz],
                                in1=tri[:jsz, :jsz],
                                op=mybir.AluOpType.mult,
                            )
                    va3 = VA[jb][:].rearrange("p (h c) -> p h c", c=D + 1)
                    for h in range(2):
                        nc.tensor.matmul(
                            pso[h][:, vo:cw],
                            va3[:jsz, 2 * ft + h, :],
                            atp[:jsz, h, vo:cw],
                            start=(n_i == 0), stop=(n_i == nblk - 1),
                        )
                # row-sum copies (DVE writes to partition 0 are legal)
                ssums = []
                for h in range(2):
                    ss = spool.tile([1, 512], NRM, name=f"ssum{h}",
                                    tag=f"ssum{h}")
                    nc.vector.tensor_copy(out=ss[:, :cw], in_=pso[h][D:D + 1, :cw])
                    ssums.append(ss)

                def norm(ft=ft, ic=ic, pso=pso, ssums=ssums, c0=c0, cw=cw):
                    for h in range(2):
                        psb = psP.tile([D, 512], F32, name="psb", tag="pp",
                                       bufs=2)
                        nc.tensor.matmul(
                            psb[:, :cw], ones64[:, :], ssums[h][:, :cw],
                            start=True, stop=True,
                        )
                        rb = spool.tile([D, 512], F32, name="rb", tag="rb",
                                        bufs=2)
                        nc.vector.reciprocal_approx_fast(
                            out=rb[:, :cw], in_=psb[:, :cw])
                        nc.vector.tensor_tensor(
                            out=AOT[ft][D * h:D * (h + 1), c0:c0 + cw],
                            in0=pso[h][0:D, :cw], in1=rb[:, :cw],
                            op=mybir.AluOpType.mult,
                        )
                pending_norm[0] = norm

            # ---------------- main schedule (chunk-outer) -----------------
            # Outer loop over chunks spreads V (per-chunk), Q/K projections
            # (per ft,chunk) and output projection (per chunk) evenly, so
            # every section has real filler covering the exp latency chain.
            for ftn in range(FT_N):
                dma_wqk(ftn)
            nb0 = min(len(JB), (R_CH[0][0] + R_CH[0][1] + P - 1) // P)
            vgen = gen_v(range(len(JB)))
            ensure(vgen, lambda: v_done[0], nb0)
            projgens = [gen_proj_ic(rc) for rc in range(len(R_CH))]
            fillers.append(projgens[0])
            fillers.append(vgen)
            fillers.append(projgens[1])
            fillers.append(projgens[2])

            for ic in range(len(R_CH)):
                c0n, cwn = R_CH[ic]
                nbn = min(len(JB), (c0n + cwn + P - 1) // P)
                ensure(vgen, lambda: v_done[0], nbn)
                for ft in range(FT_N):
                    # burst: this chunk's Q/K projection must be emitted
                    # before its attention; it doubles as boundary filler
                    ensure(projgens[ic], lambda: (ft, ic) in pc_done, True)
                    attn_chunk(ft, ic)
                    if ft == FT_N - 1:
                        # yt(ic)'s first 7 accumulations per ot only touch
                        # AOT[0..6] — pre-emit a few as cover, then the
                        # normalize, then the rest follows behind it
                        fillers.append(gen_yt(ic))
                        pump(7)
                        do_norm()
            while fillers:
                pump(10000)

    nc.compile()
    return nc


_CACHE: dict = {}


def _get_nc(causal: bool):
    if causal not in _CACHE:
        _CACHE[causal] = build2() if causal else build(causal)
    return _CACHE[causal]


def _is_causal(mask: np.ndarray) -> bool:
    if mask.shape != (S, S):
        return False
    expect = np.where(np.tril(np.ones((S, S), dtype=bool)), np.float32(0.0),
                      np.float32(NEG))
    return bool(np.array_equal(mask, expect))


MM_NP = ml_dtypes.bfloat16  # numpy dtype matching build()'s default mm_dt


def prep_inputs(x, mask, Wq, bq, Wk, Wv, bv, Wo, bo):
    """Host-side preprocessing shared by kernel() and the bench harness."""
    import ml_dtypes as mld
    scale = np.float32(1.0 / np.sqrt(D))
    xTf = np.ascontiguousarray(np.transpose(x, (0, 2, 1)).astype(np.float32))
    xT = xTf.astype(MM_NP)
    wqTf = np.ascontiguousarray((Wq.astype(np.float32) * scale).T)
    wkTf = np.ascontiguousarray(Wk.astype(np.float32).T)
    common = {
        "wqT": wqTf.astype(MM_NP),
        "wkT": wkTf.astype(MM_NP),
        "wvT": np.ascontiguousarray(Wv.astype(np.float32).T).astype(MM_NP),
        "woT": np.ascontiguousarray(Wo.astype(np.float32).T).astype(MM_NP),
        "bq": (bq.astype(np.float32) * scale),
        "bo": (bo.astype(np.float32) + Wo.astype(np.float32) @ bv.astype(np.float32)),
        # fp8 DoubleRow projection operands (scaled to dodge e4m3 denormals)
        "wq8": np.ascontiguousarray(
            (wqTf * W8SCALE).reshape(8, P, E).transpose(1, 0, 2)
        ).astype(mld.float8_e4m3fn),
        "wk8": np.ascontiguousarray(
            (wkTf * W8SCALE).reshape(8, P, E).transpose(1, 0, 2)
        ).astype(mld.float8_e4m3fn),
    }
    x8 = np.ascontiguousarray(
        xTf.reshape(B, 8, P, S).transpose(0, 2, 1, 3)
    ).astype(mld.float8_e4m3fn)
    causal = _is_causal(np.asarray(mask))
    if not causal:
        common["maskT"] = np.ascontiguousarray(np.asarray(mask, np.float32).T)
    in_maps = [dict(common, xT=xT[b], x8=x8[b]) for b in range(B)]
    return causal, in_maps


_RUNNER: dict = {}


def _get_runner(causal: bool):
    """Compile once per mask-variant; cache the jitted SPMD executable."""
    if causal in _RUNNER:
        return _RUNNER[causal]
    import jax
    from jax.sharding import Mesh, PartitionSpec, NamedSharding
    import warnings
    with warnings.catch_warnings():
        warnings.simplefilter("ignore")
        from jax.experimental.shard_map import shard_map
    from concourse import bass2jax
    from concourse.bass2jax import _bass_exec_p, install_neuronx_cc_hook

    nc = _get_nc(causal)
    install_neuronx_cc_hook()
    partition_name = (nc.partition_id_tensor.name
                      if nc.partition_id_tensor else None)
    in_names, out_names, out_avals = [], [], []
    for alloc in nc.m.functions[0].allocations:
        if not isinstance(alloc, mybir.MemoryLocationSet):
            continue
        name = alloc.memorylocations[0].name
        if alloc.kind == "ExternalInput":
            if name != partition_name:
                in_names.append(name)
        elif alloc.kind == "ExternalOutput":
            out_names.append(name)
            out_avals.append(jax.core.ShapedArray(
                tuple(alloc.tensor_shape), mybir.dt.np(alloc.dtype)))
    n_params = len(in_names)
    n_outs = len(out_names)

    def _body(*args):
        operands = list(args)
        names = list(in_names) + list(out_names)
        if partition_name is not None:
            operands.append(bass2jax.partition_id_tensor())
            names.append(partition_name)
        outs = _bass_exec_p.bind(
            *operands,
            out_avals=tuple(out_avals),
            in_names=tuple(names),
            out_names=tuple(out_names),
            lowering_input_output_aliases=(),
            sim_require_finite=True,
            sim_require_nnan=True,
            nc=nc,
        )
        return tuple(outs)

    devices = jax.devices()[:B]
    mesh = Mesh(np.asarray(devices), ("core",))
    in_specs = (PartitionSpec("core"),) * (n_params + n_outs)
    out_specs = (PartitionSpec("core"),) * n_outs
    fn = jax.jit(
        shard_map(_body, mesh=mesh, in_specs=in_specs, out_specs=out_specs,
                  check_rep=False),
        donate_argnums=tuple(range(n_params, n_params + n_outs)),
        keep_unused=True,
    )
    runner = (fn, in_names, out_names, out_avals)
    _RUNNER[causal] = runner
    return runner


def kernel(x, mask, Wq, bq, Wk, Wv, bv, Wo, bo):
    causal, in_maps = prep_inputs(x, mask, Wq, bq, Wk, Wv, bv, Wo, bo)
    fn, in_names, out_names, out_avals = _get_runner(causal)
    cat = [np.concatenate([np.asarray(m[n]) for m in in_maps], axis=0)
           for n in in_names]
    zs = [np.zeros((B * a.shape[0], *a.shape[1:]), a.dtype) for a in out_avals]
    outs = fn(*cat, *zs)
    yT = np.asarray(outs[out_names.index("yT")]).reshape(B, E, S)
    out = np.ascontiguousarray(yT.transpose(0, 2, 1).astype(np.float32))
    return out

